# revision 20
# baseline (speedup 1.0000x reference)
"""BEV distillation mask generator (CenterPoint-style gaussian max-scatter) on TRN2.

Strategy (8 NeuronCores, data-parallel): core c handles frame c//2, box-half
c%2. Max-scatter is computed in the power domain: with w = (v/c0)^32, the
per-bucket gaussian envelope max_i v_i*exp(-d_i^2/(2*sigma_b^2)) becomes
~(sum_i w_i * g_i^32)^(1/32) - a LINEAR separable convolution on the PE.

Sharding does the radius bucketing: boxes are grouped by gaussian-radius
bucket on the host (pure layout - the radius only selects which conv kernel
applies; all painting math stays on device). Each core's boxes are packed
into T tiles of 128 slots with per-bucket segments, so a tile's scatter is a
128-wide onehot matmul per covered bucket block (no 1024-wide combined
onehot, no on-device radius math):

  1. per-box params: cell via magic-number floor, u16 = (max(v,F0)*e^c)^16.
     One DVE op per tile builds BOTH onehots interleaved ([y|x] pairs vs an
     [0,0,1,1,...] iota), a second op scales both by u16 -> each scatter
     product contributes w = u16*u16.
  2. scatter: S40 accumulates per-bucket point images in PSUM [128, 8*128].
  3. per-bucket separable conv with a SINGLE kernel matrix K = g^32 * e^{+43}
     (bf16 taps in [e^-86.2, e^43]); the conv input is the image scaled by
     e^{-43} (one ACT Copy-with-scale per PSUM half per pass). Image cells
     under e^{-44.3} flush - only boxes with v < ~0.017 vanish, within the
     error floor. No hi/lo kernel split, 8 matmuls per pass.
  4. pass 2 accumulates bucket PAIRS {2,3}..{8,9} into 4 PSUM blocks; one DVE
     max-reduce over the int32 BITCAST does the 4-way max AND the int->float
     convert (max of nonneg floats == max of bits), then one ACT Exp
     evaluates c0*S^(1/32) = exp(float(bits)*ln2/(32*2^23) + bias). No sqrt
     chain; single act-table load (exp set, loaded during the DMA wait).

Host combines the two half-frame heatmaps with np.maximum -> [4,1,128,128].
Worst-case abs error vs reference ~1.42e-2 (tolerance 2e-2), dominated by
the power-domain softmax of near-equal collisions.
"""
import math

import numpy as np

FEAT = 128
BMIN, BMAX = 2, 9
NBUK = BMAX - BMIN + 1
W1024 = NBUK * 128

ALPHA = 32
LN_C0_INV = 2.669      # u = v * e^{LN_C0_INV}
F0 = 0.0046            # value floor (abs err <= F0 for tiny boxes)
S43 = 43.0             # kernel pre-scale: K = tap * e^{+43}, image * e^{-43}
EM43 = float(np.float32(math.exp(-S43)))
MAGIC = float(np.float32(8388608.0))
DELTA = float(np.float32(0.5 - 2.0 ** -18))
PAD_X = -12345.0       # pad-box x: cell lands far outside iota range

# exp recovery: heat = exp(float(bits(S)) * ln2/(32*2^23) + BIAS)
EXP_SCALE = float(np.float32(math.log(2) / (32 * (1 << 23))))
EXP_BIAS = float(np.float32(
    -LN_C0_INV - 127.0 * math.log(2) / 32.0 + 0.0861 / 64.0 * math.log(2)))

WARMUP = 34

_prog_cache = {}


def _f(x):
    return float(np.float32(x))


def _radius_buckets(bx):
    """Reference gaussian_radius -> int bucket, clamped [2, 32]. f64 numpy
    (margin to integer crossings in this data ~1e-5 >> f32/f64 delta)."""
    w = bx[:, 3] / 0.8
    h = bx[:, 4] / 0.8
    b1 = h + w
    c1 = w * h * (1.0 - 0.1) / (1.0 + 0.1)
    r1 = (b1 + np.sqrt(np.maximum(b1 * b1 - 4.0 * c1, 0.0))) / 2.0
    b2 = 2.0 * (h + w)
    c2 = (1.0 - 0.1) * w * h
    r2 = (b2 + np.sqrt(np.maximum(b2 * b2 - 16.0 * c2, 0.0))) / 2.0
    b3 = -2.0 * 0.1 * (h + w)
    c3 = (0.1 - 1.0) * w * h
    r3 = (b3 + np.sqrt(np.maximum(b3 * b3 - 16.0 * 0.1 * c3, 0.0))) / 2.0
    r = np.minimum(np.minimum(r1, r2), r3)
    valid = (w > 0) & (h > 0) & (w <= 1000) & (h <= 1000)
    ri = np.minimum(np.maximum(2, r.astype(np.int32)), 32)
    return np.where(valid, ri, 2), valid


def _plan_from_counts(maxcnt):
    """Tile/segment plan from per-bucket max (over cores) counts. Bucket
    PAIRS {2,3}..{8,9} form 4 independent groups, each packed contiguously
    and padded to a tile multiple, so every tile belongs to one group and
    each group's conv chain can start as soon as its own scatter is done."""
    starts = {}
    group_of_tile = []
    pos = 0
    for g in range(4):
        for b in (BMIN + 2 * g, BMIN + 2 * g + 1):
            starts[b] = pos
            pos += maxcnt[b]
        pos = -(-pos // 128) * 128
        while len(group_of_tile) < pos // 128:
            group_of_tile.append(g)
    total_slots = pos
    T = total_slots // 128

    tiles = []
    for t in range(T):
        lo, hi = t * 128, (t + 1) * 128
        segs = []
        for b in range(BMIN, BMAX + 1):
            s0, s1 = starts[b], starts[b] + maxcnt[b]
            a, z = max(lo, s0), min(hi, s1)
            if a < z:
                segs.append((b, a - lo, z - lo))
        tiles.append((group_of_tile[t], segs))
    return dict(starts=starts, total_slots=total_slots, T=T, tiles=tiles)


def _build_program(plan):
    import concourse.tile as tile
    from concourse import bacc, mybir

    dt = mybir.dt
    Alu = mybir.AluOpType
    Act = mybir.ActivationFunctionType
    AX = mybir.AxisListType

    T = plan["T"]
    tiles = plan["tiles"]
    group_tiles = {g: [t for t in range(T) if tiles[t][0] == g]
                   for g in range(4)}

    nc = bacc.Bacc("TRN2", target_bir_lowering=False, debug=False,
                   num_devices=8)

    # par: [x | y | off | vb | vd] each [128, T]
    par_d = nc.dram_tensor("par", [128, 5 * T], dt.float32,
                           kind="ExternalInput").ap()
    kb_d = nc.dram_tensor("kb", [128, W1024], dt.bfloat16,
                          kind="ExternalInput").ap()
    hm_d = nc.dram_tensor("hm", [128, 128], dt.float32,
                          kind="ExternalOutput").ap()

    with tile.TileContext(nc) as tc:
        with (
            tc.tile_pool(name="const", bufs=1) as cpool,
            tc.tile_pool(name="par", bufs=1) as ppool,
            tc.tile_pool(name="big", bufs=1) as bpool,
            tc.tile_pool(name="psW", bufs=1, space="PSUM") as psW,
            tc.tile_pool(name="psS", bufs=1, space="PSUM") as psS,
            tc.tile_pool(name="psM", bufs=1, space="PSUM") as psM,
            tc.tile_pool(name="psF", bufs=1, space="PSUM") as psF,
        ):
            V = nc.vector   # DVE
            A = nc.scalar   # ACT
            P = nc.gpsimd   # Pool
            PE = nc.tensor

            par = ppool.tile([128, 5 * T], dt.float32, name="par")
            nc.sync.dma_start(par[:], par_d)
            kb = cpool.tile([128, W1024], dt.bfloat16, name="kb")
            nc.sync.dma_start(kb[:], kb_d)

            # PE warmup operand first so the tensor clock starts ramping
            # immediately (p-state needs ~3us of continuous PE busy).
            wsrc = cpool.tile([128, 64], dt.float32, name="wsrc")
            P.memset(wsrc[:], 1.0)
            scr = psW.tile([64, 32], dt.float32, name="scr")
            for _ in range(WARMUP):
                PE.matmul(scr[:], wsrc[:, 0:64], wsrc[:, 32:64],
                          start=True, stop=True)

            # plain iota [128, 384] fp16 (exact ints; covers widest tile)
            iota = cpool.tile([128, 384], dt.float16, name="iota")
            P.iota(iota[:], [[1, 384]], base=0, channel_multiplier=0,
                   allow_small_or_imprecise_dtypes=True)
            bias_t = cpool.tile([128, 1], dt.float32, name="bias")
            P.memset(bias_t[:], EXP_BIAS)

            # ACT table prewarm: first ACT op is an Exp -> loads the
            # exp_and_others set once; Copy lives in the same set.
            pw = cpool.tile([128, 1], dt.float32, name="pw")
            A.activation(pw[:], bias_t[:], Act.Exp)

            x_c = par[:, 0:T]
            off_c = par[:, 2 * T:3 * T]
            vb = par[:, 3 * T:4 * T]
            vd = par[:, 4 * T:5 * T]

            # ---- per-box params (DVE; par arrival + 900ns sem gates this)
            txy = ppool.tile([128, 2 * T], dt.float32, name="txy")
            V.tensor_scalar(txy[:], par[:, 0:2 * T], _f(51.2), _f(1.25),
                            Alu.add, Alu.mult)
            t2 = ppool.tile([128, 2 * T], dt.float32, name="t2")
            V.tensor_scalar(t2[:], txy[:], DELTA, MAGIC, Alu.subtract, Alu.add)
            cxy = ppool.tile([128, 2 * T], dt.float32, name="cxy")
            V.tensor_scalar(cxy[:], t2[:], MAGIC, None, Alu.subtract)
            cy = cxy[:, T:2 * T]
            v = ppool.tile([128, T], dt.float32, name="v")
            V.scalar_tensor_tensor(v[:], vd, _f(0.5), vb, Alu.mult, Alu.add)
            j = ppool.tile([128, T], dt.float32, name="j")
            V.tensor_tensor(j[:], cxy[:, 0:T], off_c, Alu.add)
            u = ppool.tile([128, T], dt.float32, name="u")
            V.tensor_scalar(u[:], v[:], _f(F0), _f(math.exp(LN_C0_INV)),
                            Alu.max, Alu.mult)
            for _ in range(4):
                V.tensor_tensor(u[:], u[:], u[:], Alu.mult)

            # ---- scatter: 4 independent [128,256] point-image groups
            # (two groups share a PSUM bank; hazards are AP-granular) ----
            S40b = [psS.tile([128, 512], dt.float32, name=f"S40b{i}")
                    for i in range(2)]
            S40 = [S40b[g // 2][:, (g % 2) * 256:(g % 2) * 256 + 256]
                   for g in range(4)]
            seg_count = {}
            for _, segs in tiles:
                for b, *_ in segs:
                    seg_count[b] = seg_count.get(b, 0) + 1
            seen = {b: 0 for b in seg_count}

            # onehot generation: fused is_equal+mult (TensorScalarPtr runs in
            # the 4x DVE mode; TensorTensor would cap at 2x). lh of every
            # other tile goes to Pool to shorten the DVE stream.
            def gen_onehots(t, use_pool):
                g, segs = tiles[t]
                w_cols = 128 * len(segs)
                rhs = bpool.tile([128, w_cols], dt.bfloat16, name=f"rhs{t}")
                V.tensor_scalar(rhs[:], iota[:, 0:w_cols], j[:, t:t + 1],
                                u[:, t:t + 1], Alu.is_equal, Alu.mult)
                lh = bpool.tile([128, 128], dt.bfloat16, name=f"lh{t}")
                eng = P if use_pool else V
                eng.tensor_scalar(lh[:], iota[:, 0:128], cy[:, t:t + 1],
                                  u[:, t:t + 1], Alu.is_equal, Alu.mult)
                return lh, rhs

            def scatter_tile(t, lh, rhs):
                g, segs = tiles[t]
                for si, (b, c0, c1) in enumerate(segs):
                    seen[b] += 1
                    blk = (b - BMIN) % 2
                    PE.matmul(S40[g][:, blk * 128:(blk + 1) * 128],
                              lh[:], rhs[:, si * 128:(si + 1) * 128],
                              start=(seen[b] == 1),
                              stop=(seen[b] == seg_count[b]))

            # ---- conv (single kernel, image pre-scaled by e^-43) ----
            M1b = [psM.tile([128, 512], dt.float32, name=f"M1b{i}")
                   for i in range(2)]
            M1 = [M1b[g // 2][:, (g % 2) * 256:(g % 2) * 256 + 256]
                  for g in range(4)]
            S2 = psF.tile([128, 512], dt.float32, name="S2")
            # separate chunk tiles so ACT- and DVE-made copies of the last
            # group don't serialize on same-tile write tracking
            ime_c = {}
            m1e_c = {}
            for g in range(4):
                if g < 3:
                    ime_c[g] = [bpool.tile([128, 256], dt.bfloat16,
                                           name=f"ime{g}")]
                    m1e_c[g] = [bpool.tile([128, 256], dt.bfloat16,
                                           name=f"m1e{g}")]
                else:
                    ime_c[g] = [bpool.tile([128, 128], dt.bfloat16,
                                           name=f"ime{g}{c}") for c in range(2)]
                    m1e_c[g] = [bpool.tile([128, 128], dt.bfloat16,
                                           name=f"m1e{g}{c}") for c in range(2)]

            def copy_scaled(dst_parts, src, g):
                if len(dst_parts[g]) == 1:
                    A.activation(dst_parts[g][0][:], src[g][:], Act.Copy,
                                 scale=EM43)
                else:
                    A.activation(dst_parts[g][0][:], src[g][:, 0:128],
                                 Act.Copy, scale=EM43)
                    V.tensor_scalar(dst_parts[g][1][:], src[g][:, 128:256],
                                    _f(EM43), None, Alu.mult)

            def chunk_ap(parts, g, blk):
                if len(parts[g]) == 1:
                    return parts[g][0][:, blk * 128:(blk + 1) * 128]
                return parts[g][blk][:]

            def pass1(g, blk):
                b = BMIN + 2 * g + blk
                gsl = slice((b - BMIN) * 128, (b - BMIN + 1) * 128)
                PE.matmul(M1[g][:, blk * 128:(blk + 1) * 128],
                          chunk_ap(ime_c, g, blk), kb[:, gsl],
                          start=True, stop=True)

            def pass2(g, blk):
                b = BMIN + 2 * g + blk
                gsl = slice((b - BMIN) * 128, (b - BMIN + 1) * 128)
                fsl = slice(g * 128, (g + 1) * 128)
                PE.matmul(S2[:, fsl], chunk_ap(m1e_c, g, blk), kb[:, gsl],
                          start=(blk == 0), stop=(blk == 1))

            # ---- emission ----
            # ALL scatter matmuls first (the PE out-of-order window is 32
            # deep; conv matmuls emitted inside the scatter stream would
            # block tiles further ahead when their copies stall). Groups fed
            # g0..g3 so the cheapest group's chain forms the tail; its
            # copies split ACT || DVE; groups 0-2 pre-reduce off the tail.
            i_feed = 0
            for g in range(4):
                for t in group_tiles[g]:
                    lh, rhs = gen_onehots(t, use_pool=(i_feed % 2 == 1))
                    scatter_tile(t, lh, rhs)
                    i_feed += 1
            for g in range(3):
                copy_scaled(ime_c, S40, g)
                pass1(g, 0)
                pass1(g, 1)
                copy_scaled(m1e_c, M1, g)
                pass2(g, 0)
                pass2(g, 1)
            copy_scaled(ime_c, S40, 3)
            pass1(3, 0)
            pass1(3, 1)
            copy_scaled(m1e_c, M1, 3)
            pass2(3, 0)
            pass2(3, 1)

            # ---- recovery: groups 0-2 pre-reduced (hidden under group 3's
            # chain); final = max(pre, S2 block 3), then one Exp ----
            red3 = bpool.tile([128, 128], dt.int32, name="red3")
            S2i = S2[:].bitcast(dt.int32)
            red_in = type(S2i)(S2i.tensor, S2i.offset,
                               [S2i.ap[0], [1, 128], [128, 3]])
            V.tensor_reduce(red3[:], red_in, AX.X, Alu.max)
            zf = bpool.tile([128, 128], dt.float32, name="zf")
            S2b3 = S2[:, 384:512].bitcast(dt.int32)
            V.tensor_tensor(zf[:], red3[:], S2b3, Alu.max)
            out_sb = bpool.tile([128, 128], dt.float32, name="out_sb")
            A.activation(out_sb[:], zf[:], Act.Exp, scale=EXP_SCALE,
                         bias=bias_t[:])
            nc.sync.dma_start(hm_d, out_sb[:])

    nc.compile()
    return nc


def _consts():
    # banded conv kernel (single matrix, taps pre-scaled by e^{+43}),
    # bucket-major blocks b=2..9
    K = np.zeros((128, W1024), np.float32)
    for b in range(BMIN, BMAX + 1):
        ji = b - BMIN
        sig2x2 = (2 * b + 1) ** 2 / 18.0
        for dd in range(-b, b + 1):
            expo = -ALPHA * dd * dd / sig2x2 + S43
            rows = np.arange(128)
            cols = rows + dd
            ok = (cols >= 0) & (cols < 128)
            K[rows[ok], ji * 128 + cols[ok]] = math.exp(expo)
    from concourse import mybir as _mb
    bf16_t = _mb.dt.np(_mb.dt.bfloat16)
    return np.ascontiguousarray(K.astype(bf16_t))


def _shard_inputs(refined_rois, refined_scores, medium_gts, medium_scores,
                  near_unmatched, medium_unmatched):
    """Bucket-sort + pack boxes per core (pure layout/sharding). Returns
    (in_maps, plan)."""
    B = refined_rois.shape[0]
    n_rr = refined_rois.shape[1]
    n_mg = medium_gts.shape[1]
    n_nu = near_unmatched.shape[1]
    n_mu = medium_unmatched.shape[1]

    cores = []   # per core: (bxy[S,2], vb[S], vd[S], bucket[S])
    for f in range(B):
        bx = np.concatenate([refined_rois[f][:, :7], medium_gts[f][:, :7],
                             near_unmatched[f][:, :7],
                             medium_unmatched[f][:, :7]], 0).astype(np.float64)
        vbase = np.concatenate([refined_scores[f],
                                np.full(n_mg, 0.5, np.float32),
                                np.full(n_nu, 0.4, np.float32),
                                np.full(n_mu, 0.2, np.float32)]).astype(np.float32)
        cls = medium_gts[f][:, 7].astype(np.int32)
        small = (cls == 5) | (cls == 6) | (cls == 8) | (cls == 9)
        vdelta = np.zeros(n_rr + n_mg + n_nu + n_mu, np.float32)
        vdelta[n_rr:n_rr + n_mg] = np.where(small, medium_scores[f], 0.0)
        buck, _ = _radius_buckets(bx)
        idx_sorted = np.argsort(buck, kind="stable")
        for h in range(2):
            idx = idx_sorted[h::2]
            cores.append((bx[idx, 0:2].astype(np.float32), vbase[idx],
                          vdelta[idx], buck[idx]))

    maxcnt = {b: 0 for b in range(BMIN, BMAX + 1)}
    for _, _, _, bk in cores:
        cnt = np.bincount(bk, minlength=BMAX + 1)
        for b in range(BMIN, BMAX + 1):
            maxcnt[b] = max(maxcnt[b], int(cnt[b]))
    plan = _plan_from_counts(maxcnt)

    T = plan["T"]
    starts = plan["starts"]
    tiles = plan["tiles"]
    # per-slot rhs column offset: 128 * (local segment index within tile)
    slot_off = np.zeros(plan["total_slots"], np.float32)
    for t, (_, segs) in enumerate(tiles):
        for si, (b, c0, c1) in enumerate(segs):
            slot_off[t * 128 + c0:t * 128 + c1] = 128.0 * si

    kb = _consts()
    in_maps = []
    for bxy, vbs, vds, bk in cores:
        S = plan["total_slots"]
        sx = np.full(S, PAD_X, np.float32)
        sy = np.full(S, PAD_X, np.float32)
        svb = np.zeros(S, np.float32)
        svd = np.zeros(S, np.float32)
        fill = {b: starts[b] for b in range(BMIN, BMAX + 1)}
        pos = np.empty(len(bk), np.int64)
        for i, b in enumerate(bk):
            pos[i] = fill[b]
            fill[b] += 1
        sx[pos] = bxy[:, 0]
        sy[pos] = bxy[:, 1]
        svb[pos] = vbs
        svd[pos] = vds

        def lay(a):
            return a.reshape(T, 128).T

        par = np.concatenate([lay(sx), lay(sy), lay(slot_off), lay(svb),
                              lay(svd)], axis=1)
        in_maps.append(dict(par=np.ascontiguousarray(par), kb=kb))
    return in_maps, plan


def kernel(**inputs) -> np.ndarray:
    from concourse.bass_utils import run_bass_kernel_spmd

    ins = {k: np.asarray(v) for k, v in inputs.items()}
    in_maps, plan = _shard_inputs(**ins)
    key = tuple(sorted(plan["starts"].items())) + (plan["T"],)
    if _prog_cache.get("key") != key:
        _prog_cache["nc"] = _build_program(plan)
        _prog_cache["key"] = key
    nc = _prog_cache["nc"]

    res = run_bass_kernel_spmd(nc, in_maps, core_ids=list(range(8)))
    B = ins["refined_rois"].shape[0]
    out = np.empty((B, 1, FEAT, FEAT), np.float32)
    for f in range(B):
        out[f, 0] = np.maximum(res.results[2 * f]["hm"],
                               res.results[2 * f + 1]["hm"])
    return out


# revision 26
# speedup vs baseline: 1.1086x; 1.1086x over previous
"""BEV distillation mask generator (CenterPoint-style gaussian max-scatter) on TRN2.

Strategy (8 NeuronCores, data-parallel): core c handles frame c//2, box-half
c%2. Max-scatter is computed in the power domain: with w = (v/c0)^32, the
per-bucket gaussian envelope max_i v_i*exp(-d_i^2/(2*sigma_b^2)) becomes
~(sum_i w_i * g_i^32)^(1/32) - a LINEAR separable convolution on the PE.

Sharding does the radius bucketing: boxes are grouped by gaussian-radius
bucket on the host (pure layout - the radius only selects which conv kernel
applies; all painting math stays on device). Each core's boxes are packed
into T tiles of 128 slots with per-bucket segments, so a tile's scatter is a
128-wide onehot matmul per covered bucket block (no 1024-wide combined
onehot, no on-device radius math):

  1. per-box params: cell via magic-number floor, u16 = (max(v,F0)*e^c)^16.
     One DVE op per tile builds BOTH onehots interleaved ([y|x] pairs vs an
     [0,0,1,1,...] iota), a second op scales both by u16 -> each scatter
     product contributes w = u16*u16.
  2. scatter: S40 accumulates per-bucket point images in PSUM [128, 8*128].
  3. per-bucket separable conv with a SINGLE kernel matrix K = g^32 * e^{+43}
     (bf16 taps in [e^-86.2, e^43]); the conv input is the image scaled by
     e^{-43} (one ACT Copy-with-scale per PSUM half per pass). Image cells
     under e^{-44.3} flush - only boxes with v < ~0.017 vanish, within the
     error floor. No hi/lo kernel split, 8 matmuls per pass.
  4. pass 2 accumulates bucket PAIRS {2,3}..{8,9} into 4 PSUM blocks; one DVE
     max-reduce over the int32 BITCAST does the 4-way max AND the int->float
     convert (max of nonneg floats == max of bits), then one ACT Exp
     evaluates c0*S^(1/32) = exp(float(bits)*ln2/(32*2^23) + bias). No sqrt
     chain; single act-table load (exp set, loaded during the DMA wait).

Host combines the two half-frame heatmaps with np.maximum -> [4,1,128,128].
Worst-case abs error vs reference ~1.42e-2 (tolerance 2e-2), dominated by
the power-domain softmax of near-equal collisions.
"""
import math

import numpy as np

FEAT = 128
BMIN, BMAX = 2, 9
NBUK = BMAX - BMIN + 1
W1024 = NBUK * 128

ALPHA = 32
LN_C0_INV = 2.669      # u = v * e^{LN_C0_INV}
F0 = 0.0046            # value floor (abs err <= F0 for tiny boxes)
S43 = 43.0             # kernel pre-scale: K = tap * e^{+43}, image * e^{-43}
EM43 = float(np.float32(math.exp(-S43)))
MAGIC = float(np.float32(8388608.0))
DELTA = float(np.float32(0.5 - 2.0 ** -18))
PAD_X = -12345.0       # pad-box x: cell lands far outside iota range

# exp recovery: heat = exp(float(bits(S)) * ln2/(32*2^23) + BIAS)
EXP_SCALE = float(np.float32(math.log(2) / (32 * (1 << 23))))
EXP_BIAS = float(np.float32(
    -LN_C0_INV - 127.0 * math.log(2) / 32.0 + 0.0861 / 64.0 * math.log(2)))

WARMUP = 34

_prog_cache = {}


def _f(x):
    return float(np.float32(x))


def _radius_buckets(bx):
    """Reference gaussian_radius -> int bucket, clamped [2, 32]. f64 numpy
    (margin to integer crossings in this data ~1e-5 >> f32/f64 delta)."""
    w = bx[:, 3] / 0.8
    h = bx[:, 4] / 0.8
    b1 = h + w
    c1 = w * h * (1.0 - 0.1) / (1.0 + 0.1)
    r1 = (b1 + np.sqrt(np.maximum(b1 * b1 - 4.0 * c1, 0.0))) / 2.0
    b2 = 2.0 * (h + w)
    c2 = (1.0 - 0.1) * w * h
    r2 = (b2 + np.sqrt(np.maximum(b2 * b2 - 16.0 * c2, 0.0))) / 2.0
    b3 = -2.0 * 0.1 * (h + w)
    c3 = (0.1 - 1.0) * w * h
    r3 = (b3 + np.sqrt(np.maximum(b3 * b3 - 16.0 * 0.1 * c3, 0.0))) / 2.0
    r = np.minimum(np.minimum(r1, r2), r3)
    valid = (w > 0) & (h > 0) & (w <= 1000) & (h <= 1000)
    ri = np.minimum(np.maximum(2, r.astype(np.int32)), 32)
    return np.where(valid, ri, 2), valid


def _plan_from_counts(maxcnt):
    """Tile/segment plan from per-bucket max (over cores) counts. Bucket
    PAIRS {2,3}..{8,9} form 4 independent groups, each packed contiguously
    and padded to a tile multiple, so every tile belongs to one group and
    each group's conv chain can start as soon as its own scatter is done."""
    starts = {}
    group_of_tile = []
    pos = 0
    for g in range(4):
        for b in (BMIN + 2 * g, BMIN + 2 * g + 1):
            starts[b] = pos
            pos += maxcnt[b]
        pos = -(-pos // 128) * 128
        while len(group_of_tile) < pos // 128:
            group_of_tile.append(g)
    total_slots = pos
    T = total_slots // 128

    tiles = []
    for t in range(T):
        lo, hi = t * 128, (t + 1) * 128
        segs = []
        for b in range(BMIN, BMAX + 1):
            s0, s1 = starts[b], starts[b] + maxcnt[b]
            a, z = max(lo, s0), min(hi, s1)
            if a < z:
                segs.append((b, a - lo, z - lo))
        tiles.append((group_of_tile[t], segs))
    return dict(starts=starts, total_slots=total_slots, T=T, tiles=tiles)


def _build_program(plan):
    import concourse.tile as tile
    from concourse import bacc, mybir

    dt = mybir.dt
    Alu = mybir.AluOpType
    Act = mybir.ActivationFunctionType
    AX = mybir.AxisListType

    T = plan["T"]
    tiles = plan["tiles"]
    group_tiles = {g: [t for t in range(T) if tiles[t][0] == g]
                   for g in range(4)}

    nc = bacc.Bacc("TRN2", target_bir_lowering=False, debug=False,
                   num_devices=8)

    # par: [x | y | off | vb | vd] each [128, T]
    par_d = nc.dram_tensor("par", [128, 5 * T], dt.float32,
                           kind="ExternalInput").ap()
    kb_d = nc.dram_tensor("kb", [128, W1024], dt.bfloat16,
                          kind="ExternalInput").ap()
    hm_d = nc.dram_tensor("hm", [128, 128], dt.float32,
                          kind="ExternalOutput").ap()

    with tile.TileContext(nc) as tc:
        with (
            tc.tile_pool(name="const", bufs=1) as cpool,
            tc.tile_pool(name="par", bufs=1) as ppool,
            tc.tile_pool(name="big", bufs=1) as bpool,
            tc.tile_pool(name="psS", bufs=1, space="PSUM") as psS,
            tc.tile_pool(name="psF", bufs=1, space="PSUM") as psF,
        ):
            V = nc.vector   # DVE
            A = nc.scalar   # ACT
            P = nc.gpsimd   # Pool
            PE = nc.tensor

            par = ppool.tile([128, 5 * T], dt.float32, name="par")
            nc.sync.dma_start(par[:], par_d)
            kb = cpool.tile([128, W1024], dt.bfloat16, name="kb")
            nc.sync.dma_start(kb[:], kb_d)

            # PE warmup operand first so the tensor clock starts ramping
            # immediately (p-state needs ~3us of continuous PE busy). The
            # warmup scratch aliases into the S2 bank (pass2's start=True
            # resets it long after the warmup is done).
            S2 = psF.tile([128, 512], dt.float32, name="S2")
            wsrc = cpool.tile([128, 64], dt.float32, name="wsrc")
            P.memset(wsrc[:], 1.0)
            scr = S2[0:64, 0:32]
            for _ in range(WARMUP):
                PE.matmul(scr, wsrc[:, 0:64], wsrc[:, 32:64],
                          start=True, stop=True)

            # plain iota [128, 384] fp16 (exact ints; covers widest tile)
            iota = cpool.tile([128, 384], dt.float16, name="iota")
            P.iota(iota[:], [[1, 384]], base=0, channel_multiplier=0,
                   allow_small_or_imprecise_dtypes=True)
            bias_t = cpool.tile([128, 1], dt.float32, name="bias")
            P.memset(bias_t[:], EXP_BIAS)

            # ACT table prewarm: first ACT op is an Exp -> loads the
            # exp_and_others set once; Copy lives in the same set.
            pw = cpool.tile([128, 1], dt.float32, name="pw")
            A.activation(pw[:], bias_t[:], Act.Exp)

            x_c = par[:, 0:T]
            off_c = par[:, 2 * T:3 * T]
            vb = par[:, 3 * T:4 * T]
            vd = par[:, 4 * T:5 * T]

            # ---- per-box params (DVE; par arrival + 900ns sem gates this)
            txy = ppool.tile([128, 2 * T], dt.float32, name="txy")
            V.tensor_scalar(txy[:], par[:, 0:2 * T], _f(51.2), _f(1.25),
                            Alu.add, Alu.mult)
            t2 = ppool.tile([128, 2 * T], dt.float32, name="t2")
            V.tensor_scalar(t2[:], txy[:], DELTA, MAGIC, Alu.subtract, Alu.add)
            cxy = ppool.tile([128, 2 * T], dt.float32, name="cxy")
            V.tensor_scalar(cxy[:], t2[:], MAGIC, None, Alu.subtract)
            cy = cxy[:, T:2 * T]
            v = ppool.tile([128, T], dt.float32, name="v")
            V.scalar_tensor_tensor(v[:], vd, _f(0.5), vb, Alu.mult, Alu.add)
            j = ppool.tile([128, T], dt.float32, name="j")
            V.tensor_tensor(j[:], cxy[:, 0:T], off_c, Alu.add)
            u = ppool.tile([128, T], dt.float32, name="u")
            V.tensor_scalar(u[:], v[:], _f(F0), _f(math.exp(LN_C0_INV)),
                            Alu.max, Alu.mult)
            for _ in range(4):
                V.tensor_tensor(u[:], u[:], u[:], Alu.mult)

            # ---- scatter: 4 independent [128,256] point-image groups, one
            # PSUM bank each; the bank is REUSED as the group's M1 (pass1
            # starts after the ime copy reads it - a true dependency) ----
            S40 = [psS.tile([128, 256], dt.float32, name=f"S40_{g}")
                   for g in range(4)]
            seg_count = {}
            for _, segs in tiles:
                for b, *_ in segs:
                    seg_count[b] = seg_count.get(b, 0) + 1
            seen = {b: 0 for b in seg_count}

            # onehot generation: fused is_equal+mult (TensorScalarPtr runs in
            # the 4x DVE mode; TensorTensor would cap at 2x). lh of every
            # other tile goes to Pool to shorten the DVE stream.
            def gen_onehots(t, use_pool):
                g, segs = tiles[t]
                w_cols = 128 * len(segs)
                rhs = bpool.tile([128, w_cols], dt.bfloat16, name=f"rhs{t}")
                V.tensor_scalar(rhs[:], iota[:, 0:w_cols], j[:, t:t + 1],
                                u[:, t:t + 1], Alu.is_equal, Alu.mult)
                lh = bpool.tile([128, 128], dt.bfloat16, name=f"lh{t}")
                eng = P if use_pool else V
                eng.tensor_scalar(lh[:], iota[:, 0:128], cy[:, t:t + 1],
                                  u[:, t:t + 1], Alu.is_equal, Alu.mult)
                return lh, rhs

            def scatter_tile(t, lh, rhs):
                g, segs = tiles[t]
                for si, (b, c0, c1) in enumerate(segs):
                    seen[b] += 1
                    blk = (b - BMIN) % 2
                    PE.matmul(S40[g][:, blk * 128:(blk + 1) * 128],
                              lh[:], rhs[:, si * 128:(si + 1) * 128],
                              start=(seen[b] == 1),
                              stop=(seen[b] == seg_count[b]))

            # ---- conv (single kernel, image pre-scaled by e^-43) ----
            M1 = [S40[g][:] for g in range(4)]   # bank reuse
            # separate chunk tiles so ACT- and DVE-made copies of the last
            # group don't serialize on same-tile write tracking
            ime_c = {}
            m1e_c = {}
            for g in range(4):
                if g < 3:
                    ime_c[g] = [bpool.tile([128, 256], dt.bfloat16,
                                           name=f"ime{g}")]
                    m1e_c[g] = [bpool.tile([128, 256], dt.bfloat16,
                                           name=f"m1e{g}")]
                else:
                    ime_c[g] = [bpool.tile([128, 128], dt.bfloat16,
                                           name=f"ime{g}{c}") for c in range(2)]
                    m1e_c[g] = [bpool.tile([128, 128], dt.bfloat16,
                                           name=f"m1e{g}{c}") for c in range(2)]

            def copy_scaled(dst_parts, src, g, eng=None):
                if len(dst_parts[g]) == 1:
                    if eng is V:
                        V.tensor_scalar(dst_parts[g][0][:], src[g][:],
                                        _f(EM43), None, Alu.mult)
                    else:
                        A.activation(dst_parts[g][0][:], src[g][:], Act.Copy,
                                     scale=EM43)
                else:
                    A.activation(dst_parts[g][0][:], src[g][:, 0:128],
                                 Act.Copy, scale=EM43)
                    V.tensor_scalar(dst_parts[g][1][:], src[g][:, 128:256],
                                    _f(EM43), None, Alu.mult)

            def chunk_ap(parts, g, blk):
                if len(parts[g]) == 1:
                    return parts[g][0][:, blk * 128:(blk + 1) * 128]
                return parts[g][blk][:]

            def pass1(g, blk):
                b = BMIN + 2 * g + blk
                gsl = slice((b - BMIN) * 128, (b - BMIN + 1) * 128)
                PE.matmul(M1[g][:, blk * 128:(blk + 1) * 128],
                          chunk_ap(ime_c, g, blk), kb[:, gsl],
                          start=True, stop=True)

            def pass2(g, blk):
                b = BMIN + 2 * g + blk
                gsl = slice((b - BMIN) * 128, (b - BMIN + 1) * 128)
                fsl = slice(g * 128, (g + 1) * 128)
                PE.matmul(S2[:, fsl], chunk_ap(m1e_c, g, blk), kb[:, gsl],
                          start=(blk == 0), stop=(blk == 1))

            # ---- emission ----
            # ALL scatter matmuls first (the PE out-of-order window is 32
            # deep; conv matmuls emitted inside the scatter stream would
            # block tiles further ahead when their copies stall). Groups fed
            # g0..g3 so the cheapest group's chain forms the tail; its
            # copies split ACT || DVE; groups 0-2 pre-reduce off the tail.
            i_feed = 0
            for g in range(4):
                for t in group_tiles[g]:
                    lh, rhs = gen_onehots(t, use_pool=(i_feed % 2 == 1))
                    scatter_tile(t, lh, rhs)
                    i_feed += 1
            # copy engines: ACT serves g0, g1; DVE serves g2 (right after its
            # feed drains); g3 (the tail) splits ACT || DVE
            for g in range(3):
                eng = V if g == 2 else A
                copy_scaled(ime_c, S40, g, eng)
                pass1(g, 0)
                pass1(g, 1)
                copy_scaled(m1e_c, M1, g, eng)
                pass2(g, 0)
                pass2(g, 1)
            copy_scaled(ime_c, S40, 3)
            pass1(3, 0)
            pass1(3, 1)
            copy_scaled(m1e_c, M1, 3)
            pass2(3, 0)
            pass2(3, 1)

            # ---- recovery: groups 0-2 pre-reduced (hidden under group 3's
            # chain); final = max(pre, S2 block 3), then one Exp ----
            red3 = bpool.tile([128, 128], dt.int32, name="red3")
            S2i = S2[:].bitcast(dt.int32)
            red_in = type(S2i)(S2i.tensor, S2i.offset,
                               [S2i.ap[0], [1, 128], [128, 3]])
            V.tensor_reduce(red3[:], red_in, AX.X, Alu.max)
            zf = bpool.tile([128, 128], dt.float32, name="zf")
            S2b3 = S2[:, 384:512].bitcast(dt.int32)
            V.tensor_tensor(zf[:], red3[:], S2b3, Alu.max)
            out_sb = bpool.tile([128, 128], dt.float32, name="out_sb")
            A.activation(out_sb[:], zf[:], Act.Exp, scale=EXP_SCALE,
                         bias=bias_t[:])
            nc.sync.dma_start(hm_d, out_sb[:])

    nc.compile()
    return nc


def _consts():
    # banded conv kernel (single matrix, taps pre-scaled by e^{+43}),
    # bucket-major blocks b=2..9
    K = np.zeros((128, W1024), np.float32)
    for b in range(BMIN, BMAX + 1):
        ji = b - BMIN
        sig2x2 = (2 * b + 1) ** 2 / 18.0
        for dd in range(-b, b + 1):
            expo = -ALPHA * dd * dd / sig2x2 + S43
            rows = np.arange(128)
            cols = rows + dd
            ok = (cols >= 0) & (cols < 128)
            K[rows[ok], ji * 128 + cols[ok]] = math.exp(expo)
    from concourse import mybir as _mb
    bf16_t = _mb.dt.np(_mb.dt.bfloat16)
    return np.ascontiguousarray(K.astype(bf16_t))


def _shard_inputs(refined_rois, refined_scores, medium_gts, medium_scores,
                  near_unmatched, medium_unmatched):
    """Bucket-sort + pack boxes per core (pure layout/sharding). Returns
    (in_maps, plan)."""
    B = refined_rois.shape[0]
    n_rr = refined_rois.shape[1]
    n_mg = medium_gts.shape[1]
    n_nu = near_unmatched.shape[1]
    n_mu = medium_unmatched.shape[1]

    cores = []   # per core: (bxy[S,2], vb[S], vd[S], bucket[S])
    for f in range(B):
        bx = np.concatenate([refined_rois[f][:, :7], medium_gts[f][:, :7],
                             near_unmatched[f][:, :7],
                             medium_unmatched[f][:, :7]], 0).astype(np.float64)
        vbase = np.concatenate([refined_scores[f],
                                np.full(n_mg, 0.5, np.float32),
                                np.full(n_nu, 0.4, np.float32),
                                np.full(n_mu, 0.2, np.float32)]).astype(np.float32)
        cls = medium_gts[f][:, 7].astype(np.int32)
        small = (cls == 5) | (cls == 6) | (cls == 8) | (cls == 9)
        vdelta = np.zeros(n_rr + n_mg + n_nu + n_mu, np.float32)
        vdelta[n_rr:n_rr + n_mg] = np.where(small, medium_scores[f], 0.0)
        buck, _ = _radius_buckets(bx)
        idx_sorted = np.argsort(buck, kind="stable")
        for h in range(2):
            idx = idx_sorted[h::2]
            cores.append((bx[idx, 0:2].astype(np.float32), vbase[idx],
                          vdelta[idx], buck[idx]))

    maxcnt = {b: 0 for b in range(BMIN, BMAX + 1)}
    for _, _, _, bk in cores:
        cnt = np.bincount(bk, minlength=BMAX + 1)
        for b in range(BMIN, BMAX + 1):
            maxcnt[b] = max(maxcnt[b], int(cnt[b]))
    plan = _plan_from_counts(maxcnt)

    T = plan["T"]
    starts = plan["starts"]
    tiles = plan["tiles"]
    # per-slot rhs column offset: 128 * (local segment index within tile)
    slot_off = np.zeros(plan["total_slots"], np.float32)
    for t, (_, segs) in enumerate(tiles):
        for si, (b, c0, c1) in enumerate(segs):
            slot_off[t * 128 + c0:t * 128 + c1] = 128.0 * si

    kb = _consts()
    in_maps = []
    for bxy, vbs, vds, bk in cores:
        S = plan["total_slots"]
        sx = np.full(S, PAD_X, np.float32)
        sy = np.full(S, PAD_X, np.float32)
        svb = np.zeros(S, np.float32)
        svd = np.zeros(S, np.float32)
        fill = {b: starts[b] for b in range(BMIN, BMAX + 1)}
        pos = np.empty(len(bk), np.int64)
        for i, b in enumerate(bk):
            pos[i] = fill[b]
            fill[b] += 1
        sx[pos] = bxy[:, 0]
        sy[pos] = bxy[:, 1]
        svb[pos] = vbs
        svd[pos] = vds

        def lay(a):
            return a.reshape(T, 128).T

        par = np.concatenate([lay(sx), lay(sy), lay(slot_off), lay(svb),
                              lay(svd)], axis=1)
        in_maps.append(dict(par=np.ascontiguousarray(par), kb=kb))
    return in_maps, plan


def kernel(**inputs) -> np.ndarray:
    from concourse.bass_utils import run_bass_kernel_spmd

    ins = {k: np.asarray(v) for k, v in inputs.items()}
    in_maps, plan = _shard_inputs(**ins)
    key = tuple(sorted(plan["starts"].items())) + (plan["T"],)
    if _prog_cache.get("key") != key:
        _prog_cache["nc"] = _build_program(plan)
        _prog_cache["key"] = key
    nc = _prog_cache["nc"]

    res = run_bass_kernel_spmd(nc, in_maps, core_ids=list(range(8)))
    B = ins["refined_rois"].shape[0]
    out = np.empty((B, 1, FEAT, FEAT), np.float32)
    for f in range(B):
        out[f, 0] = np.maximum(res.results[2 * f]["hm"],
                               res.results[2 * f + 1]["hm"])
    return out


# revision 27
# speedup vs baseline: 1.1527x; 1.0398x over previous
"""BEV distillation mask generator (CenterPoint-style gaussian max-scatter) on TRN2.

Strategy (8 NeuronCores, data-parallel): core c handles frame c//2, box-half
c%2. Max-scatter is computed in the power domain: with w = (v/c0)^32, the
per-bucket gaussian envelope max_i v_i*exp(-d_i^2/(2*sigma_b^2)) becomes
~(sum_i w_i * g_i^32)^(1/32) - a LINEAR separable convolution on the PE.

Sharding does the radius bucketing: boxes are grouped by gaussian-radius
bucket on the host (pure layout - the radius only selects which conv kernel
applies; all painting math stays on device). Each core's boxes are packed
into T tiles of 128 slots with per-bucket segments, so a tile's scatter is a
128-wide onehot matmul per covered bucket block (no 1024-wide combined
onehot, no on-device radius math):

  1. per-box params: cell via magic-number floor, u16 = (max(v,F0)*e^c)^16.
     One DVE op per tile builds BOTH onehots interleaved ([y|x] pairs vs an
     [0,0,1,1,...] iota), a second op scales both by u16 -> each scatter
     product contributes w = u16*u16.
  2. scatter: S40 accumulates per-bucket point images in PSUM [128, 8*128].
  3. per-bucket separable conv with a SINGLE kernel matrix K = g^32 * e^{+43}
     (bf16 taps in [e^-86.2, e^43]); the conv input is the image scaled by
     e^{-43} (one ACT Copy-with-scale per PSUM half per pass). Image cells
     under e^{-44.3} flush - only boxes with v < ~0.017 vanish, within the
     error floor. No hi/lo kernel split, 8 matmuls per pass.
  4. pass 2 accumulates bucket PAIRS {2,3}..{8,9} into 4 PSUM blocks; one DVE
     max-reduce over the int32 BITCAST does the 4-way max AND the int->float
     convert (max of nonneg floats == max of bits), then one ACT Exp
     evaluates c0*S^(1/32) = exp(float(bits)*ln2/(32*2^23) + bias). No sqrt
     chain; single act-table load (exp set, loaded during the DMA wait).

Host combines the two half-frame heatmaps with np.maximum -> [4,1,128,128].
Worst-case abs error vs reference ~1.42e-2 (tolerance 2e-2), dominated by
the power-domain softmax of near-equal collisions.
"""
import math

import numpy as np

FEAT = 128
BMIN, BMAX = 2, 9
NBUK = BMAX - BMIN + 1
W1024 = NBUK * 128

ALPHA = 32
LN_C0_INV = 2.669      # u = v * e^{LN_C0_INV}
F0 = 0.0046            # value floor (abs err <= F0 for tiny boxes)
S43 = 43.0             # kernel pre-scale: K = tap * e^{+43}, image * e^{-43}
EM43 = float(np.float32(math.exp(-S43)))
MAGIC = float(np.float32(8388608.0))
DELTA = float(np.float32(0.5 - 2.0 ** -18))
PAD_X = -12345.0       # pad-box x: cell lands far outside iota range

# exp recovery: heat = exp(float(bits(S)) * ln2/(32*2^23) + BIAS)
EXP_SCALE = float(np.float32(math.log(2) / (32 * (1 << 23))))
EXP_BIAS = float(np.float32(
    -LN_C0_INV - 127.0 * math.log(2) / 32.0 + 0.0861 / 64.0 * math.log(2)))

WARMUP = 34

_prog_cache = {}


def _f(x):
    return float(np.float32(x))


def _radius_buckets(bx):
    """Reference gaussian_radius -> int bucket, clamped [2, 32]. f64 numpy
    (margin to integer crossings in this data ~1e-5 >> f32/f64 delta)."""
    w = bx[:, 3] / 0.8
    h = bx[:, 4] / 0.8
    b1 = h + w
    c1 = w * h * (1.0 - 0.1) / (1.0 + 0.1)
    r1 = (b1 + np.sqrt(np.maximum(b1 * b1 - 4.0 * c1, 0.0))) / 2.0
    b2 = 2.0 * (h + w)
    c2 = (1.0 - 0.1) * w * h
    r2 = (b2 + np.sqrt(np.maximum(b2 * b2 - 16.0 * c2, 0.0))) / 2.0
    b3 = -2.0 * 0.1 * (h + w)
    c3 = (0.1 - 1.0) * w * h
    r3 = (b3 + np.sqrt(np.maximum(b3 * b3 - 16.0 * 0.1 * c3, 0.0))) / 2.0
    r = np.minimum(np.minimum(r1, r2), r3)
    valid = (w > 0) & (h > 0) & (w <= 1000) & (h <= 1000)
    ri = np.minimum(np.maximum(2, r.astype(np.int32)), 32)
    return np.where(valid, ri, 2), valid


def _plan_from_counts(maxcnt):
    """Tile/segment plan from per-bucket max (over cores) counts. Bucket
    PAIRS {2,3}..{8,9} form 4 independent groups, each packed contiguously
    and padded to a tile multiple, so every tile belongs to one group and
    each group's conv chain can start as soon as its own scatter is done."""
    starts = {}
    group_of_tile = []
    pos = 0
    for g in range(4):
        for b in (BMIN + 2 * g, BMIN + 2 * g + 1):
            starts[b] = pos
            pos += maxcnt[b]
        pos = -(-pos // 128) * 128
        while len(group_of_tile) < pos // 128:
            group_of_tile.append(g)
    total_slots = pos
    T = total_slots // 128

    tiles = []
    for t in range(T):
        lo, hi = t * 128, (t + 1) * 128
        segs = []
        for b in range(BMIN, BMAX + 1):
            s0, s1 = starts[b], starts[b] + maxcnt[b]
            a, z = max(lo, s0), min(hi, s1)
            if a < z:
                segs.append((b, a - lo, z - lo))
        tiles.append((group_of_tile[t], segs))
    return dict(starts=starts, total_slots=total_slots, T=T, tiles=tiles)


def _build_program(plan):
    import concourse.tile as tile
    from concourse import bacc, mybir

    dt = mybir.dt
    Alu = mybir.AluOpType
    Act = mybir.ActivationFunctionType
    AX = mybir.AxisListType

    T = plan["T"]
    tiles = plan["tiles"]
    group_tiles = {g: [t for t in range(T) if tiles[t][0] == g]
                   for g in range(4)}

    nc = bacc.Bacc("TRN2", target_bir_lowering=False, debug=False,
                   num_devices=8)

    # par: [x | y | off | vb | vd] each [128, T]
    par_d = nc.dram_tensor("par", [128, 5 * T], dt.float32,
                           kind="ExternalInput").ap()
    kb_d = nc.dram_tensor("kb", [128, W1024], dt.bfloat16,
                          kind="ExternalInput").ap()
    hm_d = nc.dram_tensor("hm", [128, 128], dt.float32,
                          kind="ExternalOutput").ap()

    with tile.TileContext(nc) as tc:
        with (
            tc.tile_pool(name="const", bufs=1) as cpool,
            tc.tile_pool(name="par", bufs=1) as ppool,
            tc.tile_pool(name="big", bufs=1) as bpool,
            tc.tile_pool(name="psS", bufs=1, space="PSUM") as psS,
            tc.tile_pool(name="psF", bufs=1, space="PSUM") as psF,
        ):
            V = nc.vector   # DVE
            A = nc.scalar   # ACT
            P = nc.gpsimd   # Pool
            PE = nc.tensor

            par = ppool.tile([128, 5 * T], dt.float32, name="par")
            nc.sync.dma_start(par[:], par_d)
            kb = cpool.tile([128, W1024], dt.bfloat16, name="kb")
            nc.sync.dma_start(kb[:], kb_d)

            # PE warmup operand first so the tensor clock starts ramping
            # immediately (p-state needs ~3us of continuous PE busy). The
            # warmup scratch aliases into the S2 bank (pass2's start=True
            # resets it long after the warmup is done).
            S2 = psF.tile([128, 512], dt.float32, name="S2")
            wsrc = cpool.tile([128, 64], dt.float32, name="wsrc")
            P.memset(wsrc[:], 1.0)
            scr = S2[0:64, 0:32]
            for _ in range(WARMUP):
                PE.matmul(scr, wsrc[:, 0:64], wsrc[:, 32:64],
                          start=True, stop=True)

            # plain iota [128, 384] fp16 (exact ints; covers widest tile)
            iota = cpool.tile([128, 384], dt.float16, name="iota")
            P.iota(iota[:], [[1, 384]], base=0, channel_multiplier=0,
                   allow_small_or_imprecise_dtypes=True)
            bias_t = cpool.tile([128, 1], dt.float32, name="bias")
            P.memset(bias_t[:], EXP_BIAS)

            # ACT table prewarm: first ACT op is an Exp -> loads the
            # exp_and_others set once; Copy lives in the same set.
            pw = cpool.tile([128, 1], dt.float32, name="pw")
            A.activation(pw[:], bias_t[:], Act.Exp)

            x_c = par[:, 0:T]
            off_c = par[:, 2 * T:3 * T]
            vb = par[:, 3 * T:4 * T]
            vd = par[:, 4 * T:5 * T]

            # ---- per-box params (DVE; par arrival + 900ns sem gates this)
            txy = ppool.tile([128, 2 * T], dt.float32, name="txy")
            V.tensor_scalar(txy[:], par[:, 0:2 * T], _f(51.2), _f(1.25),
                            Alu.add, Alu.mult)
            t2 = ppool.tile([128, 2 * T], dt.float32, name="t2")
            V.tensor_scalar(t2[:], txy[:], DELTA, MAGIC, Alu.subtract, Alu.add)
            cxy = ppool.tile([128, 2 * T], dt.float32, name="cxy")
            V.tensor_scalar(cxy[:], t2[:], MAGIC, None, Alu.subtract)
            cy = cxy[:, T:2 * T]
            v = ppool.tile([128, T], dt.float32, name="v")
            V.scalar_tensor_tensor(v[:], vd, _f(0.5), vb, Alu.mult, Alu.add)
            j = ppool.tile([128, T], dt.float32, name="j")
            V.tensor_tensor(j[:], cxy[:, 0:T], off_c, Alu.add)
            u = ppool.tile([128, T], dt.float32, name="u")
            V.tensor_scalar(u[:], v[:], _f(F0), _f(math.exp(LN_C0_INV)),
                            Alu.max, Alu.mult)
            for _ in range(4):
                V.tensor_tensor(u[:], u[:], u[:], Alu.mult)

            # ---- scatter: 4 independent [128,256] point-image groups, one
            # PSUM bank each; the bank is REUSED as the group's M1 (pass1
            # starts after the ime copy reads it - a true dependency) ----
            S40 = [psS.tile([128, 256], dt.float32, name=f"S40_{g}")
                   for g in range(4)]
            seg_count = {}
            for _, segs in tiles:
                for b, *_ in segs:
                    seg_count[b] = seg_count.get(b, 0) + 1
            seen = {b: 0 for b in seg_count}

            # onehot generation: fused is_equal+mult (TensorScalarPtr runs in
            # the 4x DVE mode; TensorTensor would cap at 2x). lh of every
            # other tile goes to Pool to shorten the DVE stream.
            def gen_onehots(t, use_pool):
                g, segs = tiles[t]
                w_cols = 128 * len(segs)
                rhs = bpool.tile([128, w_cols], dt.bfloat16, name=f"rhs{t}")
                V.tensor_scalar(rhs[:], iota[:, 0:w_cols], j[:, t:t + 1],
                                u[:, t:t + 1], Alu.is_equal, Alu.mult)
                lh = bpool.tile([128, 128], dt.bfloat16, name=f"lh{t}")
                eng = P if use_pool else V
                eng.tensor_scalar(lh[:], iota[:, 0:128], cy[:, t:t + 1],
                                  u[:, t:t + 1], Alu.is_equal, Alu.mult)
                return lh, rhs

            def scatter_tile(t, lh, rhs):
                g, segs = tiles[t]
                for si, (b, c0, c1) in enumerate(segs):
                    seen[b] += 1
                    blk = (b - BMIN) % 2
                    PE.matmul(S40[g][:, blk * 128:(blk + 1) * 128],
                              lh[:], rhs[:, si * 128:(si + 1) * 128],
                              start=(seen[b] == 1),
                              stop=(seen[b] == seg_count[b]))

            # ---- conv (single kernel, image pre-scaled by e^-43) ----
            M1 = [S40[g][:] for g in range(4)]   # bank reuse
            # separate chunk tiles so ACT- and DVE-made copies of the last
            # group don't serialize on same-tile write tracking
            ime_c = {}
            m1e_c = {}
            for g in range(4):
                if g < 3:
                    ime_c[g] = [bpool.tile([128, 256], dt.bfloat16,
                                           name=f"ime{g}")]
                    m1e_c[g] = [bpool.tile([128, 256], dt.bfloat16,
                                           name=f"m1e{g}")]
                else:
                    ime_c[g] = [bpool.tile([128, 128], dt.bfloat16,
                                           name=f"ime{g}{c}") for c in range(2)]
                    m1e_c[g] = [bpool.tile([128, 128], dt.bfloat16,
                                           name=f"m1e{g}{c}") for c in range(2)]

            def copy_scaled(dst_parts, src, g, eng=None):
                if len(dst_parts[g]) == 1:
                    if eng is V:
                        V.tensor_scalar(dst_parts[g][0][:], src[g][:],
                                        _f(EM43), None, Alu.mult)
                    else:
                        A.activation(dst_parts[g][0][:], src[g][:], Act.Copy,
                                     scale=EM43)
                else:
                    A.activation(dst_parts[g][0][:], src[g][:, 0:128],
                                 Act.Copy, scale=EM43)
                    V.tensor_scalar(dst_parts[g][1][:], src[g][:, 128:256],
                                    _f(EM43), None, Alu.mult)

            def chunk_ap(parts, g, blk):
                if len(parts[g]) == 1:
                    return parts[g][0][:, blk * 128:(blk + 1) * 128]
                return parts[g][blk][:]

            def pass1(g, blk):
                b = BMIN + 2 * g + blk
                gsl = slice((b - BMIN) * 128, (b - BMIN + 1) * 128)
                PE.matmul(M1[g][:, blk * 128:(blk + 1) * 128],
                          chunk_ap(ime_c, g, blk), kb[:, gsl],
                          start=True, stop=True)

            def pass2(g, blk):
                b = BMIN + 2 * g + blk
                gsl = slice((b - BMIN) * 128, (b - BMIN + 1) * 128)
                fsl = slice(g * 128, (g + 1) * 128)
                PE.matmul(S2[:, fsl], chunk_ap(m1e_c, g, blk), kb[:, gsl],
                          start=(blk == 0), stop=(blk == 1))

            # ---- emission ----
            # ALL scatter matmuls first (the PE out-of-order window is 32
            # deep; conv matmuls emitted inside the scatter stream would
            # block tiles further ahead when their copies stall). Groups fed
            # g0..g3 so the cheapest group's chain forms the tail; its
            # copies split ACT || DVE; groups 0-2 pre-reduce off the tail.
            i_feed = 0
            for g in range(4):
                for t in group_tiles[g]:
                    lh, rhs = gen_onehots(t, use_pool=(i_feed % 2 == 1))
                    scatter_tile(t, lh, rhs)
                    i_feed += 1
            # copy engines: ACT serves g0, g1; DVE serves g2 (right after its
            # feed drains); g3 (the tail) splits ACT || DVE. Emission order =
            # engine FIFO order, sorted by expected operand readiness.
            copy_scaled(ime_c, S40, 0, A)
            pass1(0, 0)
            pass1(0, 1)
            copy_scaled(ime_c, S40, 1, A)
            pass1(1, 0)
            pass1(1, 1)
            copy_scaled(m1e_c, M1, 0, A)
            pass2(0, 0)
            pass2(0, 1)
            copy_scaled(ime_c, S40, 2, V)
            pass1(2, 0)
            pass1(2, 1)
            copy_scaled(ime_c, S40, 3)
            pass1(3, 0)
            pass1(3, 1)
            copy_scaled(m1e_c, M1, 1, A)
            pass2(1, 0)
            pass2(1, 1)
            copy_scaled(m1e_c, M1, 2, V)
            pass2(2, 0)
            pass2(2, 1)
            copy_scaled(m1e_c, M1, 3)
            pass2(3, 0)
            pass2(3, 1)

            # ---- recovery: groups 0-2 pre-reduced (hidden under group 3's
            # chain); final = max(pre, S2 block 3), then one Exp ----
            red3 = bpool.tile([128, 128], dt.int32, name="red3")
            S2i = S2[:].bitcast(dt.int32)
            red_in = type(S2i)(S2i.tensor, S2i.offset,
                               [S2i.ap[0], [1, 128], [128, 3]])
            V.tensor_reduce(red3[:], red_in, AX.X, Alu.max)
            zf = bpool.tile([128, 128], dt.float32, name="zf")
            S2b3 = S2[:, 384:512].bitcast(dt.int32)
            V.tensor_tensor(zf[:], red3[:], S2b3, Alu.max)
            out_sb = bpool.tile([128, 128], dt.float32, name="out_sb")
            A.activation(out_sb[:], zf[:], Act.Exp, scale=EXP_SCALE,
                         bias=bias_t[:])
            nc.sync.dma_start(hm_d, out_sb[:])

    nc.compile()
    return nc


def _consts():
    # banded conv kernel (single matrix, taps pre-scaled by e^{+43}),
    # bucket-major blocks b=2..9
    K = np.zeros((128, W1024), np.float32)
    for b in range(BMIN, BMAX + 1):
        ji = b - BMIN
        sig2x2 = (2 * b + 1) ** 2 / 18.0
        for dd in range(-b, b + 1):
            expo = -ALPHA * dd * dd / sig2x2 + S43
            rows = np.arange(128)
            cols = rows + dd
            ok = (cols >= 0) & (cols < 128)
            K[rows[ok], ji * 128 + cols[ok]] = math.exp(expo)
    from concourse import mybir as _mb
    bf16_t = _mb.dt.np(_mb.dt.bfloat16)
    return np.ascontiguousarray(K.astype(bf16_t))


def _shard_inputs(refined_rois, refined_scores, medium_gts, medium_scores,
                  near_unmatched, medium_unmatched):
    """Bucket-sort + pack boxes per core (pure layout/sharding). Returns
    (in_maps, plan)."""
    B = refined_rois.shape[0]
    n_rr = refined_rois.shape[1]
    n_mg = medium_gts.shape[1]
    n_nu = near_unmatched.shape[1]
    n_mu = medium_unmatched.shape[1]

    cores = []   # per core: (bxy[S,2], vb[S], vd[S], bucket[S])
    for f in range(B):
        bx = np.concatenate([refined_rois[f][:, :7], medium_gts[f][:, :7],
                             near_unmatched[f][:, :7],
                             medium_unmatched[f][:, :7]], 0).astype(np.float64)
        vbase = np.concatenate([refined_scores[f],
                                np.full(n_mg, 0.5, np.float32),
                                np.full(n_nu, 0.4, np.float32),
                                np.full(n_mu, 0.2, np.float32)]).astype(np.float32)
        cls = medium_gts[f][:, 7].astype(np.int32)
        small = (cls == 5) | (cls == 6) | (cls == 8) | (cls == 9)
        vdelta = np.zeros(n_rr + n_mg + n_nu + n_mu, np.float32)
        vdelta[n_rr:n_rr + n_mg] = np.where(small, medium_scores[f], 0.0)
        buck, _ = _radius_buckets(bx)
        idx_sorted = np.argsort(buck, kind="stable")
        for h in range(2):
            idx = idx_sorted[h::2]
            cores.append((bx[idx, 0:2].astype(np.float32), vbase[idx],
                          vdelta[idx], buck[idx]))

    maxcnt = {b: 0 for b in range(BMIN, BMAX + 1)}
    for _, _, _, bk in cores:
        cnt = np.bincount(bk, minlength=BMAX + 1)
        for b in range(BMIN, BMAX + 1):
            maxcnt[b] = max(maxcnt[b], int(cnt[b]))
    plan = _plan_from_counts(maxcnt)

    T = plan["T"]
    starts = plan["starts"]
    tiles = plan["tiles"]
    # per-slot rhs column offset: 128 * (local segment index within tile)
    slot_off = np.zeros(plan["total_slots"], np.float32)
    for t, (_, segs) in enumerate(tiles):
        for si, (b, c0, c1) in enumerate(segs):
            slot_off[t * 128 + c0:t * 128 + c1] = 128.0 * si

    kb = _consts()
    in_maps = []
    for bxy, vbs, vds, bk in cores:
        S = plan["total_slots"]
        sx = np.full(S, PAD_X, np.float32)
        sy = np.full(S, PAD_X, np.float32)
        svb = np.zeros(S, np.float32)
        svd = np.zeros(S, np.float32)
        fill = {b: starts[b] for b in range(BMIN, BMAX + 1)}
        pos = np.empty(len(bk), np.int64)
        for i, b in enumerate(bk):
            pos[i] = fill[b]
            fill[b] += 1
        sx[pos] = bxy[:, 0]
        sy[pos] = bxy[:, 1]
        svb[pos] = vbs
        svd[pos] = vds

        def lay(a):
            return a.reshape(T, 128).T

        par = np.concatenate([lay(sx), lay(sy), lay(slot_off), lay(svb),
                              lay(svd)], axis=1)
        in_maps.append(dict(par=np.ascontiguousarray(par), kb=kb))
    return in_maps, plan


def kernel(**inputs) -> np.ndarray:
    from concourse.bass_utils import run_bass_kernel_spmd

    ins = {k: np.asarray(v) for k, v in inputs.items()}
    in_maps, plan = _shard_inputs(**ins)
    key = tuple(sorted(plan["starts"].items())) + (plan["T"],)
    if _prog_cache.get("key") != key:
        _prog_cache["nc"] = _build_program(plan)
        _prog_cache["key"] = key
    nc = _prog_cache["nc"]

    res = run_bass_kernel_spmd(nc, in_maps, core_ids=list(range(8)))
    B = ins["refined_rois"].shape[0]
    out = np.empty((B, 1, FEAT, FEAT), np.float32)
    for f in range(B):
        out[f, 0] = np.maximum(res.results[2 * f]["hm"],
                               res.results[2 * f + 1]["hm"])
    return out


# revision 29
# speedup vs baseline: 1.2095x; 1.0493x over previous
"""BEV distillation mask generator (CenterPoint-style gaussian max-scatter) on TRN2.

Strategy (8 NeuronCores, data-parallel): core c handles frame c//2, box-half
c%2. Max-scatter is computed in the power domain: with w = (v/c0)^32, the
per-bucket gaussian envelope max_i v_i*exp(-d_i^2/(2*sigma_b^2)) becomes
~(sum_i w_i * g_i^32)^(1/32) - a LINEAR separable convolution on the PE.

Sharding does the radius bucketing: boxes are grouped by gaussian-radius
bucket on the host (pure layout - the radius only selects which conv kernel
applies; all painting math stays on device). Each core's boxes are packed
into T tiles of 128 slots with per-bucket segments, so a tile's scatter is a
128-wide onehot matmul per covered bucket block (no 1024-wide combined
onehot, no on-device radius math):

  1. per-box params: cell via magic-number floor, u16 = (max(v,F0)*e^c)^16.
     One DVE op per tile builds BOTH onehots interleaved ([y|x] pairs vs an
     [0,0,1,1,...] iota), a second op scales both by u16 -> each scatter
     product contributes w = u16*u16.
  2. scatter: S40 accumulates per-bucket point images in PSUM [128, 8*128].
  3. per-bucket separable conv with a SINGLE kernel matrix K = g^32 * e^{+43}
     (bf16 taps in [e^-86.2, e^43]); the conv input is the image scaled by
     e^{-43} (one ACT Copy-with-scale per PSUM half per pass). Image cells
     under e^{-44.3} flush - only boxes with v < ~0.017 vanish, within the
     error floor. No hi/lo kernel split, 8 matmuls per pass.
  4. pass 2 accumulates bucket PAIRS {2,3}..{8,9} into 4 PSUM blocks; one DVE
     max-reduce over the int32 BITCAST does the 4-way max AND the int->float
     convert (max of nonneg floats == max of bits), then one ACT Exp
     evaluates c0*S^(1/32) = exp(float(bits)*ln2/(32*2^23) + bias). No sqrt
     chain; single act-table load (exp set, loaded during the DMA wait).

Host combines the two half-frame heatmaps with np.maximum -> [4,1,128,128].
Worst-case abs error vs reference ~1.42e-2 (tolerance 2e-2), dominated by
the power-domain softmax of near-equal collisions.
"""
import math

import numpy as np

FEAT = 128
BMIN, BMAX = 2, 9
NBUK = BMAX - BMIN + 1
W1024 = NBUK * 128

ALPHA = 32
LN_C0_INV = 2.669      # u = v * e^{LN_C0_INV}
F0 = 0.0046            # value floor (abs err <= F0 for tiny boxes)
S43 = 43.0             # kernel pre-scale: K = tap * e^{+43}, image * e^{-43}
EM43 = float(np.float32(math.exp(-S43)))
MAGIC = float(np.float32(8388608.0))
DELTA = float(np.float32(0.5 - 2.0 ** -18))
PAD_X = -12345.0       # pad-box x: cell lands far outside iota range

# exp recovery: heat = exp(float(bits(S)) * ln2/(32*2^23) + BIAS)
EXP_SCALE = float(np.float32(math.log(2) / (32 * (1 << 23))))
EXP_BIAS = float(np.float32(
    -LN_C0_INV - 127.0 * math.log(2) / 32.0 + 0.0861 / 64.0 * math.log(2)))

WARMUP = 34

_prog_cache = {}


def _f(x):
    return float(np.float32(x))


def _radius_buckets(bx):
    """Reference gaussian_radius -> int bucket, clamped [2, 32]. f64 numpy
    (margin to integer crossings in this data ~1e-5 >> f32/f64 delta)."""
    w = bx[:, 3] / 0.8
    h = bx[:, 4] / 0.8
    b1 = h + w
    c1 = w * h * (1.0 - 0.1) / (1.0 + 0.1)
    r1 = (b1 + np.sqrt(np.maximum(b1 * b1 - 4.0 * c1, 0.0))) / 2.0
    b2 = 2.0 * (h + w)
    c2 = (1.0 - 0.1) * w * h
    r2 = (b2 + np.sqrt(np.maximum(b2 * b2 - 16.0 * c2, 0.0))) / 2.0
    b3 = -2.0 * 0.1 * (h + w)
    c3 = (0.1 - 1.0) * w * h
    r3 = (b3 + np.sqrt(np.maximum(b3 * b3 - 16.0 * 0.1 * c3, 0.0))) / 2.0
    r = np.minimum(np.minimum(r1, r2), r3)
    valid = (w > 0) & (h > 0) & (w <= 1000) & (h <= 1000)
    ri = np.minimum(np.maximum(2, r.astype(np.int32)), 32)
    return np.where(valid, ri, 2), valid


def _plan_from_counts(maxcnt):
    """Tile/segment plan from per-bucket max (over cores) counts. Bucket
    PAIRS {2,3}..{8,9} form 4 independent groups, each packed contiguously
    and padded to a tile multiple, so every tile belongs to one group and
    each group's conv chain can start as soon as its own scatter is done."""
    starts = {}
    group_of_tile = []
    pos = 0
    for g in range(4):
        for b in (BMIN + 2 * g, BMIN + 2 * g + 1):
            starts[b] = pos
            pos += maxcnt[b]
        pos = -(-pos // 128) * 128
        while len(group_of_tile) < pos // 128:
            group_of_tile.append(g)
    total_slots = pos
    T = total_slots // 128

    tiles = []
    for t in range(T):
        lo, hi = t * 128, (t + 1) * 128
        segs = []
        for b in range(BMIN, BMAX + 1):
            s0, s1 = starts[b], starts[b] + maxcnt[b]
            a, z = max(lo, s0), min(hi, s1)
            if a < z:
                segs.append((b, a - lo, z - lo))
        tiles.append((group_of_tile[t], segs))
    return dict(starts=starts, total_slots=total_slots, T=T, tiles=tiles)


def _build_program(plan):
    import concourse.tile as tile
    from concourse import bacc, mybir

    dt = mybir.dt
    Alu = mybir.AluOpType
    Act = mybir.ActivationFunctionType
    AX = mybir.AxisListType

    T = plan["T"]
    tiles = plan["tiles"]
    group_tiles = {g: [t for t in range(T) if tiles[t][0] == g]
                   for g in range(4)}

    nc = bacc.Bacc("TRN2", target_bir_lowering=False, debug=False,
                   num_devices=8)

    # par: [x | y | off | vb | vd] each [128, T]
    par_d = nc.dram_tensor("par", [128, 5 * T], dt.float32,
                           kind="ExternalInput").ap()
    kb_d = nc.dram_tensor("kb", [128, W1024], dt.bfloat16,
                          kind="ExternalInput").ap()
    hm_d = nc.dram_tensor("hm", [128, 128], dt.float32,
                          kind="ExternalOutput").ap()

    with tile.TileContext(nc) as tc:
        with (
            tc.tile_pool(name="const", bufs=1) as cpool,
            tc.tile_pool(name="par", bufs=1) as ppool,
            tc.tile_pool(name="big", bufs=1) as bpool,
            tc.tile_pool(name="psS", bufs=1, space="PSUM") as psS,
            tc.tile_pool(name="psF", bufs=1, space="PSUM") as psF,
        ):
            V = nc.vector   # DVE
            A = nc.scalar   # ACT
            P = nc.gpsimd   # Pool
            PE = nc.tensor

            par = ppool.tile([128, 5 * T], dt.float32, name="par")
            nc.sync.dma_start(par[:], par_d)
            kb = cpool.tile([128, W1024], dt.bfloat16, name="kb")
            nc.sync.dma_start(kb[:], kb_d)

            # PE warmup operand first so the tensor clock starts ramping
            # immediately (p-state needs ~3us of continuous PE busy). The
            # warmup scratch aliases into the S2 bank (pass2's start=True
            # resets it long after the warmup is done).
            S2 = psF.tile([128, 512], dt.float32, name="S2")
            wsrc = cpool.tile([128, 64], dt.float32, name="wsrc")
            P.memset(wsrc[:], 1.0)
            scr = S2[0:64, 0:32]
            for _ in range(WARMUP):
                PE.matmul(scr, wsrc[:, 0:64], wsrc[:, 32:64],
                          start=True, stop=True)

            # plain iota [128, 384] fp16 (exact ints; covers widest tile)
            iota = cpool.tile([128, 384], dt.float16, name="iota")
            P.iota(iota[:], [[1, 384]], base=0, channel_multiplier=0,
                   allow_small_or_imprecise_dtypes=True)
            bias_t = cpool.tile([128, 1], dt.float32, name="bias")
            P.memset(bias_t[:], EXP_BIAS)

            # ACT table prewarm: first ACT op is an Exp -> loads the
            # exp_and_others set once; Copy lives in the same set.
            pw = cpool.tile([128, 1], dt.float32, name="pw")
            A.activation(pw[:], bias_t[:], Act.Exp)

            x_c = par[:, 0:T]
            off_c = par[:, 2 * T:3 * T]
            vb = par[:, 3 * T:4 * T]
            vd = par[:, 4 * T:5 * T]

            # ---- per-box params (DVE; par arrival + 900ns sem gates this)
            txy = ppool.tile([128, 2 * T], dt.float32, name="txy")
            V.tensor_scalar(txy[:], par[:, 0:2 * T], _f(51.2), _f(1.25),
                            Alu.add, Alu.mult)
            t2 = ppool.tile([128, 2 * T], dt.float32, name="t2")
            V.tensor_scalar(t2[:], txy[:], DELTA, MAGIC, Alu.subtract, Alu.add)
            cxy = ppool.tile([128, 2 * T], dt.float32, name="cxy")
            V.tensor_scalar(cxy[:], t2[:], MAGIC, None, Alu.subtract)
            cy = cxy[:, T:2 * T]
            v = ppool.tile([128, T], dt.float32, name="v")
            V.scalar_tensor_tensor(v[:], vd, _f(0.5), vb, Alu.mult, Alu.add)
            j = ppool.tile([128, T], dt.float32, name="j")
            V.tensor_tensor(j[:], cxy[:, 0:T], off_c, Alu.add)
            u = ppool.tile([128, T], dt.float32, name="u")
            V.tensor_scalar(u[:], v[:], _f(F0), _f(math.exp(LN_C0_INV)),
                            Alu.max, Alu.mult)
            for _ in range(4):
                V.tensor_tensor(u[:], u[:], u[:], Alu.mult)

            # ---- scatter: 4 independent [128,256] point-image groups, one
            # PSUM bank each; the bank is REUSED as the group's M1 (pass1
            # starts after the ime copy reads it - a true dependency) ----
            S40 = [psS.tile([128, 256], dt.float32, name=f"S40_{g}")
                   for g in range(4)]
            seg_count = {}
            for _, segs in tiles:
                for b, *_ in segs:
                    seg_count[b] = seg_count.get(b, 0) + 1
            seen = {b: 0 for b in seg_count}

            # onehot generation: fused is_equal+mult (TensorScalarPtr runs in
            # the 4x DVE mode; TensorTensor would cap at 2x). lh of every
            # other tile goes to Pool to shorten the DVE stream.
            def gen_onehots(t, use_pool):
                g, segs = tiles[t]
                w_cols = 128 * len(segs)
                rhs = bpool.tile([128, w_cols], dt.bfloat16, name=f"rhs{t}")
                V.tensor_scalar(rhs[:], iota[:, 0:w_cols], j[:, t:t + 1],
                                u[:, t:t + 1], Alu.is_equal, Alu.mult)
                lh = bpool.tile([128, 128], dt.bfloat16, name=f"lh{t}")
                eng = P if use_pool else V
                eng.tensor_scalar(lh[:], iota[:, 0:128], cy[:, t:t + 1],
                                  u[:, t:t + 1], Alu.is_equal, Alu.mult)
                return lh, rhs

            def scatter_tile(t, lh, rhs):
                g, segs = tiles[t]
                for si, (b, c0, c1) in enumerate(segs):
                    seen[b] += 1
                    blk = (b - BMIN) % 2
                    PE.matmul(S40[g][:, blk * 128:(blk + 1) * 128],
                              lh[:], rhs[:, si * 128:(si + 1) * 128],
                              start=(seen[b] == 1),
                              stop=(seen[b] == seg_count[b]))

            # ---- conv (single kernel, image pre-scaled by e^-43) ----
            M1 = [S40[g][:] for g in range(4)]   # bank reuse
            # separate chunk tiles so ACT- and DVE-made copies of the last
            # group don't serialize on same-tile write tracking
            ime_c = {g: [bpool.tile([128, 256], dt.bfloat16, name=f"ime{g}")]
                     for g in range(4)}
            m1e_c = {g: [bpool.tile([128, 256], dt.bfloat16, name=f"m1e{g}")]
                     for g in range(4)}

            def copy_scaled(dst_parts, src, g, eng=None):
                if len(dst_parts[g]) == 1:
                    if eng is V:
                        V.tensor_scalar(dst_parts[g][0][:], src[g][:],
                                        _f(EM43), None, Alu.mult)
                    else:
                        A.activation(dst_parts[g][0][:], src[g][:], Act.Copy,
                                     scale=EM43)
                else:
                    A.activation(dst_parts[g][0][:], src[g][:, 0:128],
                                 Act.Copy, scale=EM43)
                    V.tensor_scalar(dst_parts[g][1][:], src[g][:, 128:256],
                                    _f(EM43), None, Alu.mult)

            def chunk_ap(parts, g, blk):
                if len(parts[g]) == 1:
                    return parts[g][0][:, blk * 128:(blk + 1) * 128]
                return parts[g][blk][:]

            def pass1(g, blk):
                b = BMIN + 2 * g + blk
                gsl = slice((b - BMIN) * 128, (b - BMIN + 1) * 128)
                PE.matmul(M1[g][:, blk * 128:(blk + 1) * 128],
                          chunk_ap(ime_c, g, blk), kb[:, gsl],
                          start=True, stop=True)

            def pass2(g, blk):
                b = BMIN + 2 * g + blk
                gsl = slice((b - BMIN) * 128, (b - BMIN + 1) * 128)
                fsl = slice(g * 128, (g + 1) * 128)
                PE.matmul(S2[:, fsl], chunk_ap(m1e_c, g, blk), kb[:, gsl],
                          start=(blk == 0), stop=(blk == 1))

            # ---- emission ----
            # ALL scatter matmuls first (the PE out-of-order window is 32
            # deep; conv matmuls emitted inside the scatter stream would
            # block tiles further ahead when their copies stall). Groups fed
            # g0..g3 so the cheapest group's chain forms the tail; its
            # copies split ACT || DVE; groups 0-2 pre-reduce off the tail.
            i_feed = 0
            for g in range(4):
                for t in group_tiles[g]:
                    lh, rhs = gen_onehots(t, use_pool=(i_feed % 2 == 1))
                    scatter_tile(t, lh, rhs)
                    i_feed += 1
            # copy engines: ACT serves g0, g1; DVE serves g2 (right after its
            # feed drains); g3 (the tail) splits ACT || DVE. Emission order =
            # engine FIFO order, sorted by expected operand readiness.
            red3 = bpool.tile([128, 128], dt.int32, name="red3")
            zf = bpool.tile([128, 128], dt.float32, name="zf")

            copy_scaled(ime_c, S40, 0, A)
            pass1(0, 0)
            pass1(0, 1)
            copy_scaled(ime_c, S40, 1, A)
            pass1(1, 0)
            pass1(1, 1)
            copy_scaled(m1e_c, M1, 0, A)
            pass2(0, 0)
            pass2(0, 1)
            copy_scaled(ime_c, S40, 2, V)
            pass1(2, 0)
            pass1(2, 1)
            copy_scaled(ime_c, S40, 3, A)
            pass1(3, 0)
            pass1(3, 1)
            copy_scaled(m1e_c, M1, 1, A)
            pass2(1, 0)
            pass2(1, 1)
            copy_scaled(m1e_c, M1, 2, V)
            pass2(2, 0)
            pass2(2, 1)
            # groups 0-2 pre-reduced on DVE while group 3 finishes on ACT
            S2i = S2[:].bitcast(dt.int32)
            red_in = type(S2i)(S2i.tensor, S2i.offset,
                               [S2i.ap[0], [1, 128], [128, 3]])
            V.tensor_reduce(red3[:], red_in, AX.X, Alu.max)
            copy_scaled(m1e_c, M1, 3, A)
            pass2(3, 0)
            pass2(3, 1)
            S2b3 = S2[:, 384:512].bitcast(dt.int32)
            V.tensor_tensor(zf[:], red3[:], S2b3, Alu.max)
            out_sb = bpool.tile([128, 128], dt.float32, name="out_sb")
            A.activation(out_sb[:], zf[:], Act.Exp, scale=EXP_SCALE,
                         bias=bias_t[:])
            nc.sync.dma_start(hm_d, out_sb[:])

    nc.compile()
    return nc


def _consts():
    # banded conv kernel (single matrix, taps pre-scaled by e^{+43}),
    # bucket-major blocks b=2..9
    K = np.zeros((128, W1024), np.float32)
    for b in range(BMIN, BMAX + 1):
        ji = b - BMIN
        sig2x2 = (2 * b + 1) ** 2 / 18.0
        for dd in range(-b, b + 1):
            expo = -ALPHA * dd * dd / sig2x2 + S43
            rows = np.arange(128)
            cols = rows + dd
            ok = (cols >= 0) & (cols < 128)
            K[rows[ok], ji * 128 + cols[ok]] = math.exp(expo)
    from concourse import mybir as _mb
    bf16_t = _mb.dt.np(_mb.dt.bfloat16)
    return np.ascontiguousarray(K.astype(bf16_t))


def _shard_inputs(refined_rois, refined_scores, medium_gts, medium_scores,
                  near_unmatched, medium_unmatched):
    """Bucket-sort + pack boxes per core (pure layout/sharding). Returns
    (in_maps, plan)."""
    B = refined_rois.shape[0]
    n_rr = refined_rois.shape[1]
    n_mg = medium_gts.shape[1]
    n_nu = near_unmatched.shape[1]
    n_mu = medium_unmatched.shape[1]

    cores = []   # per core: (bxy[S,2], vb[S], vd[S], bucket[S])
    for f in range(B):
        bx = np.concatenate([refined_rois[f][:, :7], medium_gts[f][:, :7],
                             near_unmatched[f][:, :7],
                             medium_unmatched[f][:, :7]], 0).astype(np.float64)
        vbase = np.concatenate([refined_scores[f],
                                np.full(n_mg, 0.5, np.float32),
                                np.full(n_nu, 0.4, np.float32),
                                np.full(n_mu, 0.2, np.float32)]).astype(np.float32)
        cls = medium_gts[f][:, 7].astype(np.int32)
        small = (cls == 5) | (cls == 6) | (cls == 8) | (cls == 9)
        vdelta = np.zeros(n_rr + n_mg + n_nu + n_mu, np.float32)
        vdelta[n_rr:n_rr + n_mg] = np.where(small, medium_scores[f], 0.0)
        buck, _ = _radius_buckets(bx)
        idx_sorted = np.argsort(buck, kind="stable")
        for h in range(2):
            idx = idx_sorted[h::2]
            cores.append((bx[idx, 0:2].astype(np.float32), vbase[idx],
                          vdelta[idx], buck[idx]))

    maxcnt = {b: 0 for b in range(BMIN, BMAX + 1)}
    for _, _, _, bk in cores:
        cnt = np.bincount(bk, minlength=BMAX + 1)
        for b in range(BMIN, BMAX + 1):
            maxcnt[b] = max(maxcnt[b], int(cnt[b]))
    plan = _plan_from_counts(maxcnt)

    T = plan["T"]
    starts = plan["starts"]
    tiles = plan["tiles"]
    # per-slot rhs column offset: 128 * (local segment index within tile)
    slot_off = np.zeros(plan["total_slots"], np.float32)
    for t, (_, segs) in enumerate(tiles):
        for si, (b, c0, c1) in enumerate(segs):
            slot_off[t * 128 + c0:t * 128 + c1] = 128.0 * si

    kb = _consts()
    in_maps = []
    for bxy, vbs, vds, bk in cores:
        S = plan["total_slots"]
        sx = np.full(S, PAD_X, np.float32)
        sy = np.full(S, PAD_X, np.float32)
        svb = np.zeros(S, np.float32)
        svd = np.zeros(S, np.float32)
        fill = {b: starts[b] for b in range(BMIN, BMAX + 1)}
        pos = np.empty(len(bk), np.int64)
        for i, b in enumerate(bk):
            pos[i] = fill[b]
            fill[b] += 1
        sx[pos] = bxy[:, 0]
        sy[pos] = bxy[:, 1]
        svb[pos] = vbs
        svd[pos] = vds

        def lay(a):
            return a.reshape(T, 128).T

        par = np.concatenate([lay(sx), lay(sy), lay(slot_off), lay(svb),
                              lay(svd)], axis=1)
        in_maps.append(dict(par=np.ascontiguousarray(par), kb=kb))
    return in_maps, plan


def kernel(**inputs) -> np.ndarray:
    from concourse.bass_utils import run_bass_kernel_spmd

    ins = {k: np.asarray(v) for k, v in inputs.items()}
    in_maps, plan = _shard_inputs(**ins)
    key = tuple(sorted(plan["starts"].items())) + (plan["T"],)
    if _prog_cache.get("key") != key:
        _prog_cache["nc"] = _build_program(plan)
        _prog_cache["key"] = key
    nc = _prog_cache["nc"]

    res = run_bass_kernel_spmd(nc, in_maps, core_ids=list(range(8)))
    B = ins["refined_rois"].shape[0]
    out = np.empty((B, 1, FEAT, FEAT), np.float32)
    for f in range(B):
        out[f, 0] = np.maximum(res.results[2 * f]["hm"],
                               res.results[2 * f + 1]["hm"])
    return out


# revision 32
# speedup vs baseline: 1.2271x; 1.0146x over previous
"""BEV distillation mask generator (CenterPoint-style gaussian max-scatter) on TRN2.

Strategy (8 NeuronCores, data-parallel): core c handles frame c//2, box-half
c%2. Max-scatter is computed in the power domain: with w = (v/c0)^32, the
per-bucket gaussian envelope max_i v_i*exp(-d_i^2/(2*sigma_b^2)) becomes
~(sum_i w_i * g_i^32)^(1/32) - a LINEAR separable convolution on the PE.

Sharding does the radius bucketing: boxes are grouped by gaussian-radius
bucket on the host (pure layout - the radius only selects which conv kernel
applies; all painting math stays on device). Each core's boxes are packed
into T tiles of 128 slots with per-bucket segments, so a tile's scatter is a
128-wide onehot matmul per covered bucket block (no 1024-wide combined
onehot, no on-device radius math):

  1. per-box params: cell via magic-number floor, u16 = (max(v,F0)*e^c)^16.
     One DVE op per tile builds BOTH onehots interleaved ([y|x] pairs vs an
     [0,0,1,1,...] iota), a second op scales both by u16 -> each scatter
     product contributes w = u16*u16.
  2. scatter: S40 accumulates per-bucket point images in PSUM [128, 8*128].
  3. per-bucket separable conv with a SINGLE kernel matrix K = g^32 * e^{+43}
     (bf16 taps in [e^-86.2, e^43]); the conv input is the image scaled by
     e^{-43} (one ACT Copy-with-scale per PSUM half per pass). Image cells
     under e^{-44.3} flush - only boxes with v < ~0.017 vanish, within the
     error floor. No hi/lo kernel split, 8 matmuls per pass.
  4. pass 2 accumulates bucket PAIRS {2,3}..{8,9} into 4 PSUM blocks; one DVE
     max-reduce over the int32 BITCAST does the 4-way max AND the int->float
     convert (max of nonneg floats == max of bits), then one ACT Exp
     evaluates c0*S^(1/32) = exp(float(bits)*ln2/(32*2^23) + bias). No sqrt
     chain; single act-table load (exp set, loaded during the DMA wait).

Host combines the two half-frame heatmaps with np.maximum -> [4,1,128,128].
Worst-case abs error vs reference ~1.42e-2 (tolerance 2e-2), dominated by
the power-domain softmax of near-equal collisions.
"""
import math

import numpy as np

FEAT = 128
BMIN, BMAX = 2, 9
NBUK = BMAX - BMIN + 1
W1024 = NBUK * 128

ALPHA = 32
LN_C0_INV = 2.669      # u = v * e^{LN_C0_INV}
F0 = 0.0046            # value floor (abs err <= F0 for tiny boxes)
S43 = 43.0             # kernel pre-scale: K = tap * e^{+43}, image * e^{-43}
EM43 = float(np.float32(math.exp(-S43)))
MAGIC = float(np.float32(8388608.0))
DELTA = float(np.float32(0.5 - 2.0 ** -18))
PAD_X = -12345.0       # pad-box x: cell lands far outside iota range

# exp recovery: heat = exp(float(bits(S)) * ln2/(32*2^23) + BIAS)
EXP_SCALE = float(np.float32(math.log(2) / (32 * (1 << 23))))
EXP_BIAS = float(np.float32(
    -LN_C0_INV - 127.0 * math.log(2) / 32.0 + 0.0861 / 64.0 * math.log(2)))

WARMUP = 34

_prog_cache = {}


def _f(x):
    return float(np.float32(x))


def _radius_buckets(bx):
    """Reference gaussian_radius -> int bucket, clamped [2, 32]. f64 numpy
    (margin to integer crossings in this data ~1e-5 >> f32/f64 delta)."""
    w = bx[:, 3] / 0.8
    h = bx[:, 4] / 0.8
    b1 = h + w
    c1 = w * h * (1.0 - 0.1) / (1.0 + 0.1)
    r1 = (b1 + np.sqrt(np.maximum(b1 * b1 - 4.0 * c1, 0.0))) / 2.0
    b2 = 2.0 * (h + w)
    c2 = (1.0 - 0.1) * w * h
    r2 = (b2 + np.sqrt(np.maximum(b2 * b2 - 16.0 * c2, 0.0))) / 2.0
    b3 = -2.0 * 0.1 * (h + w)
    c3 = (0.1 - 1.0) * w * h
    r3 = (b3 + np.sqrt(np.maximum(b3 * b3 - 16.0 * 0.1 * c3, 0.0))) / 2.0
    r = np.minimum(np.minimum(r1, r2), r3)
    valid = (w > 0) & (h > 0) & (w <= 1000) & (h <= 1000)
    ri = np.minimum(np.maximum(2, r.astype(np.int32)), 32)
    return np.where(valid, ri, 2), valid


def _plan_from_counts(maxcnt):
    """Tile/segment plan from per-bucket max (over cores) counts. Bucket
    PAIRS {2,3}..{8,9} form 4 independent groups, each packed contiguously
    and padded to a tile multiple, so every tile belongs to one group and
    each group's conv chain can start as soon as its own scatter is done."""
    starts = {}
    group_of_tile = []
    pos = 0
    for g in range(4):
        for b in (BMIN + 2 * g, BMIN + 2 * g + 1):
            starts[b] = pos
            pos += maxcnt[b]
        pos = -(-pos // 128) * 128
        while len(group_of_tile) < pos // 128:
            group_of_tile.append(g)
    total_slots = pos
    T = total_slots // 128

    tiles = []
    for t in range(T):
        lo, hi = t * 128, (t + 1) * 128
        segs = []
        for b in range(BMIN, BMAX + 1):
            s0, s1 = starts[b], starts[b] + maxcnt[b]
            a, z = max(lo, s0), min(hi, s1)
            if a < z:
                segs.append((b, a - lo, z - lo))
        tiles.append((group_of_tile[t], segs))
    return dict(starts=starts, total_slots=total_slots, T=T, tiles=tiles)


def _build_program(plan):
    import concourse.tile as tile
    from concourse import bacc, mybir

    dt = mybir.dt
    Alu = mybir.AluOpType
    Act = mybir.ActivationFunctionType
    AX = mybir.AxisListType

    T = plan["T"]
    tiles = plan["tiles"]
    group_tiles = {g: [t for t in range(T) if tiles[t][0] == g]
                   for g in range(4)}

    nc = bacc.Bacc("TRN2", target_bir_lowering=False, debug=False,
                   num_devices=8)

    # par: [x | y | off | vb | vd] each [128, T]
    par_d = nc.dram_tensor("par", [128, 5 * T], dt.float32,
                           kind="ExternalInput").ap()
    kb_d = nc.dram_tensor("kb", [128, W1024], dt.bfloat16,
                          kind="ExternalInput").ap()
    hm_d = nc.dram_tensor("hm", [128, 128], dt.float32,
                          kind="ExternalOutput").ap()

    with tile.TileContext(nc) as tc:
        with (
            tc.tile_pool(name="const", bufs=1) as cpool,
            tc.tile_pool(name="par", bufs=1) as ppool,
            tc.tile_pool(name="big", bufs=1) as bpool,
            tc.tile_pool(name="psS", bufs=1, space="PSUM") as psS,
            tc.tile_pool(name="psF", bufs=1, space="PSUM") as psF,
        ):
            V = nc.vector   # DVE
            A = nc.scalar   # ACT
            P = nc.gpsimd   # Pool
            PE = nc.tensor

            par = ppool.tile([128, 5 * T], dt.float32, name="par")
            nc.sync.dma_start(par[:], par_d)
            kb = cpool.tile([128, W1024], dt.bfloat16, name="kb")
            nc.sync.dma_start(kb[:], kb_d)

            # PE warmup operand first so the tensor clock starts ramping
            # immediately (p-state needs ~3us of continuous PE busy). The
            # warmup scratch aliases into the S2a bank (pass2's start=True
            # resets it long after the warmup is done). S2 block 3 lives in
            # its own bank so the early pre-reduce of blocks 0-2 doesn't
            # false-block the tail group's pass2 write.
            S2 = psF.tile([128, 384], dt.float32, name="S2")
            S2b = psF.tile([128, 128], dt.float32, name="S2b")
            wsrc = cpool.tile([128, 64], dt.float32, name="wsrc")
            P.memset(wsrc[:], 1.0)
            scr = S2[0:64, 0:32]
            for _ in range(WARMUP):
                PE.matmul(scr, wsrc[:, 0:64], wsrc[:, 32:64],
                          start=True, stop=True)

            # plain iota [128, 384] fp16 (exact ints; covers widest tile)
            iota = cpool.tile([128, 384], dt.float16, name="iota")
            P.iota(iota[:], [[1, 384]], base=0, channel_multiplier=0,
                   allow_small_or_imprecise_dtypes=True)
            bias_t = cpool.tile([128, 1], dt.float32, name="bias")
            P.memset(bias_t[:], EXP_BIAS)

            # ACT table prewarm: first ACT op is an Exp -> loads the
            # exp_and_others set once; Copy lives in the same set.
            pw = cpool.tile([128, 1], dt.float32, name="pw")
            A.activation(pw[:], bias_t[:], Act.Exp)

            x_c = par[:, 0:T]
            off_c = par[:, 2 * T:3 * T]
            vb = par[:, 3 * T:4 * T]
            vd = par[:, 4 * T:5 * T]

            # ---- per-box params (DVE; par arrival + 900ns sem gates this)
            txy = ppool.tile([128, 2 * T], dt.float32, name="txy")
            V.tensor_scalar(txy[:], par[:, 0:2 * T], _f(51.2), _f(1.25),
                            Alu.add, Alu.mult)
            t2 = ppool.tile([128, 2 * T], dt.float32, name="t2")
            V.tensor_scalar(t2[:], txy[:], DELTA, MAGIC, Alu.subtract, Alu.add)
            cxy = ppool.tile([128, 2 * T], dt.float32, name="cxy")
            V.tensor_scalar(cxy[:], t2[:], MAGIC, None, Alu.subtract)
            cy = cxy[:, T:2 * T]
            v = ppool.tile([128, T], dt.float32, name="v")
            V.scalar_tensor_tensor(v[:], vd, _f(0.5), vb, Alu.mult, Alu.add)
            j = ppool.tile([128, T], dt.float32, name="j")
            V.tensor_tensor(j[:], cxy[:, 0:T], off_c, Alu.add)
            u = ppool.tile([128, T], dt.float32, name="u")
            V.tensor_scalar(u[:], v[:], _f(F0), _f(math.exp(LN_C0_INV)),
                            Alu.max, Alu.mult)
            for _ in range(4):
                V.tensor_tensor(u[:], u[:], u[:], Alu.mult)

            # ---- scatter: 4 independent [128,256] point-image groups, one
            # PSUM bank each; the bank is REUSED as the group's M1 (pass1
            # starts after the ime copy reads it - a true dependency) ----
            S40 = [psS.tile([128, 256], dt.float32, name=f"S40_{g}")
                   for g in range(4)]
            seg_count = {}
            for _, segs in tiles:
                for b, *_ in segs:
                    seg_count[b] = seg_count.get(b, 0) + 1
            seen = {b: 0 for b in seg_count}

            # onehot generation: fused is_equal+mult (TensorScalarPtr runs in
            # the 4x DVE mode; TensorTensor would cap at 2x). lh of every
            # other tile goes to Pool to shorten the DVE stream.
            def gen_onehots(t, use_pool):
                g, segs = tiles[t]
                w_cols = 128 * len(segs)
                rhs = bpool.tile([128, w_cols], dt.bfloat16, name=f"rhs{t}")
                V.tensor_scalar(rhs[:], iota[:, 0:w_cols], j[:, t:t + 1],
                                u[:, t:t + 1], Alu.is_equal, Alu.mult)
                lh = bpool.tile([128, 128], dt.bfloat16, name=f"lh{t}")
                eng = P if use_pool else V
                eng.tensor_scalar(lh[:], iota[:, 0:128], cy[:, t:t + 1],
                                  u[:, t:t + 1], Alu.is_equal, Alu.mult)
                return lh, rhs

            def scatter_tile(t, lh, rhs):
                g, segs = tiles[t]
                for si, (b, c0, c1) in enumerate(segs):
                    seen[b] += 1
                    blk = (b - BMIN) % 2
                    PE.matmul(S40[g][:, blk * 128:(blk + 1) * 128],
                              lh[:], rhs[:, si * 128:(si + 1) * 128],
                              start=(seen[b] == 1),
                              stop=(seen[b] == seg_count[b]))

            # ---- conv (single kernel, image pre-scaled by e^-43) ----
            M1 = [S40[g][:] for g in range(4)]   # bank reuse
            # separate chunk tiles so ACT- and DVE-made copies of the last
            # group don't serialize on same-tile write tracking
            ime_c = {g: [bpool.tile([128, 256], dt.bfloat16, name=f"ime{g}")]
                     for g in range(4)}
            m1e_c = {g: [bpool.tile([128, 256], dt.bfloat16, name=f"m1e{g}")]
                     for g in range(4)}

            def copy_scaled(dst_parts, src, g, eng=None):
                if len(dst_parts[g]) == 1:
                    if eng is V:
                        V.tensor_scalar(dst_parts[g][0][:], src[g][:],
                                        _f(EM43), None, Alu.mult)
                    else:
                        A.activation(dst_parts[g][0][:], src[g][:], Act.Copy,
                                     scale=EM43)
                else:
                    A.activation(dst_parts[g][0][:], src[g][:, 0:128],
                                 Act.Copy, scale=EM43)
                    V.tensor_scalar(dst_parts[g][1][:], src[g][:, 128:256],
                                    _f(EM43), None, Alu.mult)

            def chunk_ap(parts, g, blk):
                if len(parts[g]) == 1:
                    return parts[g][0][:, blk * 128:(blk + 1) * 128]
                return parts[g][blk][:]

            def pass1(g, blk):
                b = BMIN + 2 * g + blk
                gsl = slice((b - BMIN) * 128, (b - BMIN + 1) * 128)
                PE.matmul(M1[g][:, blk * 128:(blk + 1) * 128],
                          chunk_ap(ime_c, g, blk), kb[:, gsl],
                          start=True, stop=True)

            def pass2(g, blk):
                b = BMIN + 2 * g + blk
                gsl = slice((b - BMIN) * 128, (b - BMIN + 1) * 128)
                dst = S2b[:] if g == 3 else S2[:, g * 128:(g + 1) * 128]
                PE.matmul(dst, chunk_ap(m1e_c, g, blk), kb[:, gsl],
                          start=(blk == 0), stop=(blk == 1))

            # ---- emission ----
            # ALL scatter matmuls first (the PE out-of-order window is 32
            # deep; conv matmuls emitted inside the scatter stream would
            # block tiles further ahead when their copies stall). Groups fed
            # g0..g3 so the cheapest group's chain forms the tail; its
            # copies split ACT || DVE; groups 0-2 pre-reduce off the tail.
            i_feed = 0
            for g in range(4):
                for t in group_tiles[g]:
                    lh, rhs = gen_onehots(t, use_pool=(i_feed % 2 == 1))
                    scatter_tile(t, lh, rhs)
                    i_feed += 1
            # copy engines: ACT serves g0, g1; DVE serves g2 (right after its
            # feed drains); g3 (the tail) splits ACT || DVE. Emission order =
            # engine FIFO order, sorted by expected operand readiness.
            red3 = bpool.tile([128, 128], dt.int32, name="red3")
            zf = bpool.tile([128, 128], dt.float32, name="zf")

            copy_scaled(ime_c, S40, 0, A)
            pass1(0, 0)
            pass1(0, 1)
            copy_scaled(ime_c, S40, 1, A)
            pass1(1, 0)
            pass1(1, 1)
            copy_scaled(m1e_c, M1, 0, A)
            pass2(0, 0)
            pass2(0, 1)
            copy_scaled(ime_c, S40, 2, V)
            pass1(2, 0)
            pass1(2, 1)
            copy_scaled(ime_c, S40, 3, A)
            pass1(3, 0)
            pass1(3, 1)
            copy_scaled(m1e_c, M1, 1, A)
            pass2(1, 0)
            pass2(1, 1)
            copy_scaled(m1e_c, M1, 2, V)
            pass2(2, 0)
            pass2(2, 1)
            # groups 0-2 pre-reduced on DVE while group 3 finishes on ACT
            S2i = S2[:].bitcast(dt.int32)
            red_in = type(S2i)(S2i.tensor, S2i.offset,
                               [S2i.ap[0], [1, 128], [128, 3]])
            V.tensor_reduce(red3[:], red_in, AX.X, Alu.max)
            copy_scaled(m1e_c, M1, 3, A)
            pass2(3, 0)
            pass2(3, 1)
            V.tensor_tensor(zf[:], red3[:], S2b[:].bitcast(dt.int32), Alu.max)
            out_sb = bpool.tile([128, 128], dt.float32, name="out_sb")
            A.activation(out_sb[:], zf[:], Act.Exp, scale=EXP_SCALE,
                         bias=bias_t[:])
            nc.sync.dma_start(hm_d, out_sb[:])

    nc.compile()
    return nc


def _consts():
    # banded conv kernel (single matrix, taps pre-scaled by e^{+43}),
    # bucket-major blocks b=2..9
    K = np.zeros((128, W1024), np.float32)
    for b in range(BMIN, BMAX + 1):
        ji = b - BMIN
        sig2x2 = (2 * b + 1) ** 2 / 18.0
        for dd in range(-b, b + 1):
            expo = -ALPHA * dd * dd / sig2x2 + S43
            rows = np.arange(128)
            cols = rows + dd
            ok = (cols >= 0) & (cols < 128)
            K[rows[ok], ji * 128 + cols[ok]] = math.exp(expo)
    from concourse import mybir as _mb
    bf16_t = _mb.dt.np(_mb.dt.bfloat16)
    return np.ascontiguousarray(K.astype(bf16_t))


def _shard_inputs(refined_rois, refined_scores, medium_gts, medium_scores,
                  near_unmatched, medium_unmatched):
    """Bucket-sort + pack boxes per core (pure layout/sharding). Returns
    (in_maps, plan)."""
    B = refined_rois.shape[0]
    n_rr = refined_rois.shape[1]
    n_mg = medium_gts.shape[1]
    n_nu = near_unmatched.shape[1]
    n_mu = medium_unmatched.shape[1]

    cores = []   # per core: (bxy[S,2], vb[S], vd[S], bucket[S])
    for f in range(B):
        bx = np.concatenate([refined_rois[f][:, :7], medium_gts[f][:, :7],
                             near_unmatched[f][:, :7],
                             medium_unmatched[f][:, :7]], 0).astype(np.float64)
        vbase = np.concatenate([refined_scores[f],
                                np.full(n_mg, 0.5, np.float32),
                                np.full(n_nu, 0.4, np.float32),
                                np.full(n_mu, 0.2, np.float32)]).astype(np.float32)
        cls = medium_gts[f][:, 7].astype(np.int32)
        small = (cls == 5) | (cls == 6) | (cls == 8) | (cls == 9)
        vdelta = np.zeros(n_rr + n_mg + n_nu + n_mu, np.float32)
        vdelta[n_rr:n_rr + n_mg] = np.where(small, medium_scores[f], 0.0)
        buck, _ = _radius_buckets(bx)
        idx_sorted = np.argsort(buck, kind="stable")
        for h in range(2):
            idx = idx_sorted[h::2]
            cores.append((bx[idx, 0:2].astype(np.float32), vbase[idx],
                          vdelta[idx], buck[idx]))

    maxcnt = {b: 0 for b in range(BMIN, BMAX + 1)}
    for _, _, _, bk in cores:
        cnt = np.bincount(bk, minlength=BMAX + 1)
        for b in range(BMIN, BMAX + 1):
            maxcnt[b] = max(maxcnt[b], int(cnt[b]))
    plan = _plan_from_counts(maxcnt)

    T = plan["T"]
    starts = plan["starts"]
    tiles = plan["tiles"]
    # per-slot rhs column offset: 128 * (local segment index within tile)
    slot_off = np.zeros(plan["total_slots"], np.float32)
    for t, (_, segs) in enumerate(tiles):
        for si, (b, c0, c1) in enumerate(segs):
            slot_off[t * 128 + c0:t * 128 + c1] = 128.0 * si

    kb = _consts()
    in_maps = []
    for bxy, vbs, vds, bk in cores:
        S = plan["total_slots"]
        sx = np.full(S, PAD_X, np.float32)
        sy = np.full(S, PAD_X, np.float32)
        svb = np.zeros(S, np.float32)
        svd = np.zeros(S, np.float32)
        fill = {b: starts[b] for b in range(BMIN, BMAX + 1)}
        pos = np.empty(len(bk), np.int64)
        for i, b in enumerate(bk):
            pos[i] = fill[b]
            fill[b] += 1
        sx[pos] = bxy[:, 0]
        sy[pos] = bxy[:, 1]
        svb[pos] = vbs
        svd[pos] = vds

        def lay(a):
            return a.reshape(T, 128).T

        par = np.concatenate([lay(sx), lay(sy), lay(slot_off), lay(svb),
                              lay(svd)], axis=1)
        in_maps.append(dict(par=np.ascontiguousarray(par), kb=kb))
    return in_maps, plan


def kernel(**inputs) -> np.ndarray:
    from concourse.bass_utils import run_bass_kernel_spmd

    ins = {k: np.asarray(v) for k, v in inputs.items()}
    in_maps, plan = _shard_inputs(**ins)
    key = tuple(sorted(plan["starts"].items())) + (plan["T"],)
    if _prog_cache.get("key") != key:
        _prog_cache["nc"] = _build_program(plan)
        _prog_cache["key"] = key
    nc = _prog_cache["nc"]

    res = run_bass_kernel_spmd(nc, in_maps, core_ids=list(range(8)))
    B = ins["refined_rois"].shape[0]
    out = np.empty((B, 1, FEAT, FEAT), np.float32)
    for f in range(B):
        out[f, 0] = np.maximum(res.results[2 * f]["hm"],
                               res.results[2 * f + 1]["hm"])
    return out
